# revision 35
# baseline (speedup 1.0000x reference)
"""GQA (n_group == n_head) causal attention kernel for 8 Trainium2 NeuronCores.

Sharding: core c -> (batch b = c//2, head-half hh = c%2).  Each core computes
Q/K/V projections for its 8 heads over the full sequence, causal attention,
and a partial output projection against its 512 rows of Wo.  The host sums
the two partial outputs per batch (the tensor-parallel reduce) and
transposes back.

Device pipeline (per core), all attention operands bf16:
  QT/KT = (x @ W).T          [dout, t]   f32r matmuls, bias added in the
                                         PSUM->SBUF copy (DVE)
  V     = x @ Wv             [t, dout]   column 64 of each V tile is 1.0 so
                                         the PV matmul also accumulates the
                                         softmax denominator
  scT   = K_h @ Q_h.T        [k, q]      both heads of a pair into one
                                         2-bank PSUM tile
  expT  = exp(scT/8)                     one fused Activation per key tile
  pv    = expT.T @ [V_h | 1] [q, 65]     transposed PV: 65-column matmuls
                                         instead of 512-column ones
  ao    = pv[:, :64] / pv[:, 64]         DVE normalize into [q, hd] layout
  aoT   = transpose(ao)      [hd, q]     PE transpose via identity
  outT  = Wo_h.T @ aoT + bo  [dout, q]   partial; host adds core pairs

The attention inner loop is Activation-engine bound (exp), so projection /
out-projection / transpose work is interleaved into the attention tile
stream ("fillers") to keep the tensor engine from idling, paced by a static
cost model of both engines.
"""

import os
import collections
from collections import deque
from contextlib import ExitStack

import numpy as np

import concourse.bass as bass
import concourse.mybir as mybir
import concourse.tile as tile
from concourse import bacc
from concourse.bass import ds, ts
from concourse.bass_utils import run_bass_kernel_spmd

B, T, D = 4, 2048, 1024
H, HD = 16, 64
NCORES = 8
HH = H // 2            # heads per core = 8
DH = HH * HD           # head dims per core = 512
QC = 512               # query block (attention outer tile)
NQC = T // QC          # 4 query blocks
KT = 128               # key tile
TB = 512               # token block for projections
F32 = mybir.dt.float32
F32R = mybir.dt.float32r
BF16 = mybir.dt.bfloat16

# static engine cost estimates (ns) used only to pace filler emission
PE_NS = 1e9 / 2.4e9
ACT_NS = 1e9 / 1.2e9
ACT_OH = float(os.environ.get("KERNEL_ACT_OH", "290"))
THRESH = float(os.environ.get("KERNEL_THRESH", "650"))
TXP_INLINE = os.environ.get("KERNEL_TXP_INLINE", "0") == "1"

LAST_RESULTS = None


def _build_nc():
    nc = bacc.Bacc(
        "TRN2",
        target_bir_lowering=False,
        debug=False,
        enable_asserts=False,
        num_devices=NCORES,
    )

    xT = nc.dram_tensor("xT", [D, T], BF16, kind="ExternalInput").ap()
    wq = nc.dram_tensor("wq", [D, DH], BF16, kind="ExternalInput").ap()
    wk = nc.dram_tensor("wk", [D, DH], BF16, kind="ExternalInput").ap()
    wv = nc.dram_tensor("wv", [D, DH], BF16, kind="ExternalInput").ap()
    wo = nc.dram_tensor("wo", [DH, D], BF16, kind="ExternalInput").ap()
    bq_t = nc.dram_tensor("bq_t", [128, DH // 128], F32, kind="ExternalInput").ap()
    bk_t = nc.dram_tensor("bk_t", [128, DH // 128], F32, kind="ExternalInput").ap()
    bo_t = nc.dram_tensor("bo_t", [128, D // 128], F32, kind="ExternalInput").ap()
    tri = nc.dram_tensor("tri", [128, 128], BF16, kind="ExternalInput").ap()
    ident = nc.dram_tensor("ident", [128, 128], BF16, kind="ExternalInput").ap()
    outT = nc.dram_tensor("outT", [D, T], BF16, kind="ExternalOutput").ap()
    dbg = os.environ.get("KERNEL_DEBUG", "0") == "1"
    if dbg:
        qt_d = nc.dram_tensor("qt_d", [128, 4 * T], BF16, kind="ExternalOutput").ap()
        kt_d = nc.dram_tensor("kt_d", [128, 4 * T], BF16, kind="ExternalOutput").ap()
        v_d = nc.dram_tensor("v_d", [128, (T // KT) * HH * (HD + 1)], BF16, kind="ExternalOutput").ap()
        aoT_d = nc.dram_tensor("aoT_d", [128, 4 * T], BF16, kind="ExternalOutput").ap()

    with tile.TileContext(nc) as tc, ExitStack() as ctx:
        res = ctx.enter_context(tc.tile_pool(name="res", bufs=1))
        # resident SBUF tensors; row c*128+p of qt/kt = local dout
        qt_sb = res.tile([128, 4, T], BF16, tag="qt")
        kt_sb = res.tile([128, 4, T], BF16, tag="kt")
        v_sb = res.tile([128, T // KT, HH, HD + 1], BF16, tag="v")
        aoT_sb = res.tile([128, 4, T], BF16, tag="aoT")
        wq_sb = res.tile([128, 8, DH], BF16, tag="wq")
        wk_sb = res.tile([128, 8, DH], BF16, tag="wk")
        wv_sb = res.tile([128, 8, DH], BF16, tag="wv")
        wo_sb = res.tile([128, 4, D], BF16, tag="wo")
        tri_sb = res.tile([128, 128], BF16, tag="tri")
        id_sb = res.tile([128, 128], BF16, tag="id")
        bq_sb = res.tile([128, 4], F32, tag="bq")
        bk_sb = res.tile([128, 4], F32, tag="bk")
        bo_sb = res.tile([128, 8], F32, tag="bo")

        # PSUM: sc2 2 banks x2 + pv 2 banks x1 + mm 1 bank x2 = 8 banks
        sc2p = ctx.enter_context(tc.tile_pool(name="sc2", bufs=2, space="PSUM"))
        pvp = ctx.enter_context(tc.tile_pool(name="pvp", bufs=1, space="PSUM"))
        mm = ctx.enter_context(tc.tile_pool(name="mm", bufs=2, space="PSUM"))
        xpool = ctx.enter_context(tc.tile_pool(name="xp", bufs=2))
        # et is a ring holding every exp tile of the current (qc, hp) block:
        # PV accumulation groups must run back-to-back per PSUM bank, so PV
        # for chunk cc is deferred until its diagonal tile and then reads all
        # earlier exp tiles.  16 live tiles at qc=3 + slack for the next hp.
        etp = ctx.enter_context(tc.tile_pool(name="et", bufs=18))
        pvr = ctx.enter_context(tc.tile_pool(name="pvr", bufs=2))
        rdp = ctx.enter_context(tc.tile_pool(name="rd", bufs=2))
        stp = ctx.enter_context(tc.tile_pool(name="st", bufs=4))
        # ao tiles live only from normalize until the (qc, hp) transpose
        aop = ctx.enter_context(tc.tile_pool(name="aop", bufs=8))

        # initial DMAs: the SP queue needs ~650ns PER dma_start issue (the
        # transfers are only ~364ns), so batch chunks in pairs and issue on
        # two queues in parallel (SP: wq,wk,biases; ACT: x0,wv,tri,id)
        xts = {}
        xTv = xT.rearrange("(c p) t -> p c t", p=128)
        wqv = wq.rearrange("(c p) d -> p c d", p=128)
        wkv = wk.rearrange("(c p) d -> p c d", p=128)
        wvv = wv.rearrange("(c p) d -> p c d", p=128)

        def load_x(tb):
            xt = xpool.tile([128, 8, TB], BF16, tag="xt", name="xt")
            for c4 in range(2):
                nc.sync.dma_start(
                    out=xt[:, 4 * c4 : 4 * c4 + 4, :],
                    in_=xTv[:, 4 * c4 : 4 * c4 + 4, ts(tb, TB)],
                )
            xts[tb] = xt

        xt0 = xpool.tile([128, 8, TB], BF16, tag="xt", name="xt")
        for c2 in range(4):
            sl2 = slice(2 * c2, 2 * c2 + 2)
            nc.sync.dma_start(out=wq_sb[:, sl2, :], in_=wqv[:, sl2, :])
            nc.scalar.dma_start(out=xt0[:, sl2, :], in_=xTv[:, sl2, ts(0, TB)])
        xts[0] = xt0
        for c2 in range(4):
            sl2 = slice(2 * c2, 2 * c2 + 2)
            nc.sync.dma_start(out=wk_sb[:, sl2, :], in_=wkv[:, sl2, :])
            nc.scalar.dma_start(out=wv_sb[:, sl2, :], in_=wvv[:, sl2, :])
        nc.sync.dma_start(out=bq_sb, in_=bq_t)
        nc.sync.dma_start(out=bk_sb, in_=bk_t)
        nc.sync.dma_start(out=bo_sb, in_=bo_t)
        nc.scalar.dma_start(out=tri_sb, in_=tri)
        nc.scalar.dma_start(out=id_sb, in_=ident)
        nc.vector.memset(v_sb[:, :, :, HD : HD + 1], 1.0)
        wo_loaded = [False]

        def load_wo():
            if not wo_loaded[0]:
                wo_loaded[0] = True
                for c in range(4):
                    nc.sync.dma_start(out=wo_sb[:, c, :], in_=wo[ts(c, 128), :])

        # ---- filler machinery: projq units must land before the next query
        # block; lateq units (out-proj, transposes) have no deadline and are
        # saved for the exp-bound late blocks.
        qq = deque()      # Q units: due before the next query block starts
        projq = deque()   # K/V units: due before the next block's diagonal
        lateq = deque()
        debt = [0.0]  # accumulated ACT-over-PE time not yet filled

        stats = collections.Counter()
        where = ["init"]
        due_count = [0]  # kv units at the head of projq due this block

        def fillers(thresh=THRESH):
            while debt[0] > thresh:
                if qq:
                    qq.popleft()()
                    stats[(where[0], "qq")] += 1
                elif projq:
                    projq.popleft()()
                    if due_count[0] > 0:
                        due_count[0] -= 1
                    stats[(where[0], "kv")] += 1
                elif lateq:
                    lateq.popleft()()
                    stats[(where[0], "late")] += 1
                else:
                    stats[(where[0], "DRY")] += 1
                    break

        # units are split into ~426ns halves so a filler fits the PE-idle
        # window of one ACT-bound attention tile without delaying the next
        # QK (which would stall the saturated Activation engine). Halves are
        # adjacent in their FIFO queue, so at most one filler accumulation
        # group is ever open per PSUM bank.
        def qk_unit(tb, dt, wsb, bias, dst):
            box = []

            def a():
                ps = mm.tile([128, TB], F32, tag="mm", name="ps")
                box.append(ps)
                for c in range(4):
                    nc.tensor.matmul(ps, wsb[:, c, ts(dt, 128)],
                                     xts[tb][:, c, :], start=(c == 0), stop=False)
                debt[0] -= 4 * TB * PE_NS

            def b():
                ps = box[0]
                for c in range(4, 8):
                    nc.tensor.matmul(ps, wsb[:, c, ts(dt, 128)],
                                     xts[tb][:, c, :], start=False, stop=(c == 7))
                nc.vector.tensor_scalar_add(
                    dst[:, dt, ts(tb, TB)], ps, bias[:, dt : dt + 1]
                )
                debt[0] -= 4 * TB * PE_NS

            return [a, b]

        def v_unit(tb, tt):
            box = []

            def a():
                ps = mm.tile([128, DH], F32, tag="mm", name="ps")
                box.append(ps)
                for c in range(4):
                    nc.tensor.matmul(ps, xts[tb][:, c, ts(tt, 128)],
                                     wv_sb[:, c, :], start=(c == 0), stop=False)
                debt[0] -= 4 * DH * PE_NS

            def b():
                ps = box[0]
                for c in range(4, 8):
                    nc.tensor.matmul(ps, xts[tb][:, c, ts(tt, 128)],
                                     wv_sb[:, c, :], start=False, stop=(c == 7))
                nc.vector.tensor_copy(
                    v_sb[:, tb * (TB // 128) + tt, :, 0:HD],
                    ps.rearrange("p (h c) -> p h c", h=HH),
                )
                debt[0] -= 4 * DH * PE_NS

            return [a, b]

        def q_units(tb):
            return [h for dt in range(4)
                    for h in qk_unit(tb, dt, wq_sb, bq_sb, qt_sb)]

        def kv_units(tb):
            return [h for dt in range(4)
                    for h in qk_unit(tb, dt, wk_sb, bk_sb, kt_sb)] + [
                h for tt in range(4) for h in v_unit(tb, tt)]

        def txp_unit(qc, hp, aot):
            def emit():
                tx = mm.tile([128, 4, 128], BF16, tag="mm", name="tx")
                for cc in range(4):
                    nc.tensor.transpose(tx[:, cc, :], aot[:, cc, :], id_sb)
                nc.vector.tensor_copy(
                    aoT_sb[:, hp, ts(qc, QC)], tx.rearrange("p a b -> p (a b)")
                )
                debt[0] -= 4 * 128 * PE_NS
            return emit

        def oproj_unit(qc, dt):
            box = []

            def a():
                if qc == 3 and dt % 2 == 1:
                    ps = sc2p.tile([128, 2, QC], F32, tag="sc", name="ps")[:, 0, :]
                else:
                    ps = mm.tile([128, QC], F32, tag="mm", name="ps")
                box.append(ps)
                for hp in range(2):
                    nc.tensor.matmul(ps, wo_sb[:, hp, ts(dt, 128)],
                                     aoT_sb[:, hp, ts(qc, QC)],
                                     start=(hp == 0), stop=False)
                debt[0] -= 2 * QC * PE_NS

            def b():
                ps = box[0]
                for hp in range(2, 4):
                    nc.tensor.matmul(ps, wo_sb[:, hp, ts(dt, 128)],
                                     aoT_sb[:, hp, ts(qc, QC)],
                                     start=False, stop=(hp == 3))
                st = stp.tile([128, QC], BF16, tag="st", name="st")
                if qc == 3:
                    # tail: the Activation engine is idle once attention is
                    # done; Identity supports a per-partition bias AP
                    nc.scalar.activation(
                        st, ps, mybir.ActivationFunctionType.Identity,
                        bias=bo_sb[:, dt : dt + 1],
                    )
                else:
                    nc.vector.tensor_scalar_add(st, ps, bo_sb[:, dt : dt + 1])
                nc.sync.dma_start(out=outT[ts(dt, 128), ts(qc, QC)], in_=st)
                debt[0] -= 2 * QC * PE_NS

            return [a, b]

        # token block 0 projections, chunk-major across four concurrent
        # PSUM groups (2 in mm + 2 in sc2) so the tensor engine keeps pace
        # with the streaming weight/x DMAs instead of stalling per group
        def proj_tb0():
            xt = xts[0]
            for kind in ("q", "k", "v"):
                pa = mm.tile([128, TB], F32, tag="mm", name="pa")
                pb = mm.tile([128, TB], F32, tag="mm", name="pb")
                sc = sc2p.tile([128, 2, QC], F32, tag="sc", name="sc")
                pss = [pa, pb, sc[:, 0, :], sc[:, 1, :]]
                for c in range(8):
                    for g in range(4):
                        if kind == "q":
                            nc.tensor.matmul(pss[g], wq_sb[:, c, ts(g, 128)],
                                             xt[:, c, :], start=(c == 0), stop=(c == 7))
                        elif kind == "k":
                            nc.tensor.matmul(pss[g], wk_sb[:, c, ts(g, 128)],
                                             xt[:, c, :], start=(c == 0), stop=(c == 7))
                        else:
                            nc.tensor.matmul(pss[g], xt[:, c, ts(g, 128)],
                                             wv_sb[:, c, :], start=(c == 0), stop=(c == 7))
                for g in range(4):
                    if kind == "q":
                        if g % 2 == 0:
                            nc.vector.tensor_scalar_add(qt_sb[:, g, ts(0, TB)],
                                                        pss[g], bq_sb[:, g : g + 1])
                        else:
                            nc.scalar.activation(
                                qt_sb[:, g, ts(0, TB)], pss[g],
                                mybir.ActivationFunctionType.Identity,
                                bias=bq_sb[:, g : g + 1])
                    elif kind == "k":
                        if g % 2 == 0:
                            nc.vector.tensor_scalar_add(kt_sb[:, g, ts(0, TB)],
                                                        pss[g], bk_sb[:, g : g + 1])
                        else:
                            nc.scalar.activation(
                                kt_sb[:, g, ts(0, TB)], pss[g],
                                mybir.ActivationFunctionType.Identity,
                                bias=bk_sb[:, g : g + 1])
                    else:
                        eng = nc.vector if g % 2 == 0 else nc.scalar
                        if g % 2 == 0:
                            nc.vector.tensor_copy(
                                v_sb[:, g, :, 0:HD],
                                pss[g].rearrange("p (h c) -> p h c", h=HH))
                        else:
                            nc.scalar.activation(
                                v_sb[:, g, :, 0:HD],
                                pss[g].rearrange("p (h c) -> p h c", h=HH),
                                mybir.ActivationFunctionType.Copy)

        proj_tb0()
        debt[0] = 0.0

        for qc in range(NQC):
            due_count[0] = len(projq)  # leftovers: due at this diagonal
            if qc + 1 < NQC:
                load_x(qc + 1)
                qq.extend(q_units(qc + 1))
                projq.extend(kv_units(qc + 1))
            nkt = (qc + 1) * (QC // KT)
            for hp in range(4):
                where[0] = f"qc{qc}hp{hp}"
                # chunk stride padded to 128 floats so each 65-float accumulation
                # region stays inside one 2 KiB PSUM bank (head i -> bank i)
                pv = pvp.tile([128, 2, 4, 128], F32, tag="pv", name="pv")
                ets = []
                for kt in range(nkt):
                    j = kt - qc * (QC // KT)  # >= 0 on the diagonal block
                    q0 = j * KT if j > 0 else 0
                    qn = QC - q0
                    if j == 0 and hp == 0 and due_count[0] > 0:
                        # this block's K/V must be emitted before the first
                        # diagonal QK reads them (engines run in order)
                        while due_count[0] > 0:
                            projq.popleft()()
                            due_count[0] -= 1
                            stats[(where[0], "kv-forced")] += 1
                        debt[0] = min(debt[0], 0.0)
                    sc = sc2p.tile([128, 2, QC], F32, tag="sc", name="sc")
                    for i in range(2):
                        po = i * 64
                        nc.tensor.matmul(
                            sc[:, i, q0:],
                            kt_sb[po : po + 64, hp, ts(kt, 128)],
                            qt_sb[po : po + 64, hp, ds(qc * QC + q0, qn)],
                            start=True,
                            stop=True,
                        )
                    et = etp.tile([128, 2, QC], BF16, tag="et", name="et")
                    nc.scalar.activation(
                        et[:, :, q0:],
                        sc[:, :, q0:],
                        mybir.ActivationFunctionType.Exp,
                        scale=0.125,
                    )
                    ets.append(et)
                    pe_tile = 2 * qn * PE_NS
                    if j >= 0:
                        # causal boundary lies inside q-chunk j only; the
                        # mask is all-ones for chunks right of it
                        for i in range(2):
                            nc.vector.tensor_mul(
                                et[:, i, ts(j, KT)], et[:, i, ts(j, KT)], tri_sb
                            )
                        # chunk j's keys are complete: emit its whole PV
                        # accumulation group back-to-back (one per bank)
                        cc = j
                        for i in range(2):
                            for kk in range(kt + 1):
                                nc.tensor.matmul(
                                    pv[:, i, cc, 0 : HD + 1],
                                    ets[kk][:, i, ts(cc, 128)],
                                    v_sb[:, kk, hp * 2 + i, :],
                                    start=(kk == 0),
                                    stop=(kk == kt),
                                )
                        pe_tile += 2 * (kt + 1) * 65 * PE_NS
                    debt[0] += (2 * qn * ACT_NS + ACT_OH) - pe_tile
                    fillers()
                # drain pv quickly so the single PSUM slot frees: exact
                # denominators from PSUM, raw bf16 copy, then normalize
                rd = rdp.tile([128, 2, 4], F32, tag="rd", name="rd")
                nc.vector.reciprocal(rd, pv[:, :, :, HD : HD + 1])
                praw = pvr.tile([128, 2, 4, HD], BF16, tag="praw", name="praw")
                nc.vector.tensor_copy(praw, pv[:, :, :, 0:HD])
                aot = aop.tile([128, 4, 128], BF16, tag="ao", name="aot")
                for i in range(2):
                    for cc in range(4):
                        nc.vector.tensor_scalar_mul(
                            aot[:, cc, i * 64 : (i + 1) * 64],
                            praw[:, i, cc, :],
                            rd[:, i, cc : cc + 1],
                        )
                if TXP_INLINE:
                    txp_unit(qc, hp, aot)()
                else:
                    lateq.append(txp_unit(qc, hp, aot))
            # Q of the next block must be in before it starts; K/V only
            # before its diagonal tiles, so they keep filling the next block
            where[0] = f"qc{qc}end"
            while qq:
                qq.popleft()()
                stats[(where[0], "qq-forced")] += 1
            debt[0] = min(debt[0], 0.0)
            load_wo()
            for dt in range(8):
                lateq.extend(oproj_unit(qc, dt))
        where[0] = "tail"
        while lateq:
            lateq.popleft()()
            stats[("tail", "late")] += 1
        if os.environ.get("KERNEL_STATS", "0") == "1":
            for k in sorted(stats):
                print(f"  {k}: {stats[k]}")
        if dbg:
            nc.sync.dma_start(out=qt_d, in_=qt_sb.rearrange("p a b -> p (a b)"))
            nc.sync.dma_start(out=kt_d, in_=kt_sb.rearrange("p a b -> p (a b)"))
            nc.sync.dma_start(out=v_d, in_=v_sb.rearrange("p a b c -> p (a b c)"))
            nc.sync.dma_start(out=aoT_d, in_=aoT_sb.rearrange("p a b -> p (a b)"))

    nc.compile()
    return nc


def kernel(x, Wq, bq, Wk, bk, Wv, bv, Wo, bo):
    global LAST_RESULTS
    import ml_dtypes

    x = np.asarray(x, np.float32)
    Wq, bq = np.asarray(Wq, np.float32), np.asarray(bq, np.float32)
    Wk, bk = np.asarray(Wk, np.float32), np.asarray(bk, np.float32)
    Wv, bv = np.asarray(Wv, np.float32), np.asarray(bv, np.float32)
    Wo, bo = np.asarray(Wo, np.float32), np.asarray(bo, np.float32)

    tri = np.triu(np.ones((128, 128), np.float32)).astype(ml_dtypes.bfloat16)
    ident = np.eye(128, dtype=np.float32).astype(ml_dtypes.bfloat16)

    in_maps = []
    for c in range(NCORES):
        b, hh = c // 2, c % 2
        sl = slice(hh * DH, (hh + 1) * DH)
        # attn out includes +bv per head dim (softmax weights sum to 1), so
        # bv contributes bv_slice @ Wo_slice to this core's partial output;
        # bo itself is carried by the hh == 0 core of each pair.
        bo_eff = bv[sl] @ Wo[sl, :] + (bo if hh == 0 else 0.0)
        in_maps.append(
            {
                "xT": np.ascontiguousarray(x[b].T).astype(ml_dtypes.bfloat16),
                "wq": np.ascontiguousarray(Wq[:, sl]).astype(ml_dtypes.bfloat16),
                "wk": np.ascontiguousarray(Wk[:, sl]).astype(ml_dtypes.bfloat16),
                "wv": np.ascontiguousarray(Wv[:, sl]).astype(ml_dtypes.bfloat16),
                "wo": np.ascontiguousarray(Wo[sl, :]).astype(ml_dtypes.bfloat16),
                "bq_t": np.ascontiguousarray(bq[sl].reshape(4, 128).T),
                "bk_t": np.ascontiguousarray(bk[sl].reshape(4, 128).T),
                "bo_t": np.ascontiguousarray(bo_eff.reshape(8, 128).T),
                "tri": tri,
                "ident": ident,
            }
        )

    nc = _build_nc()
    res = run_bass_kernel_spmd(
        nc,
        in_maps,
        core_ids=list(range(NCORES)),
        trace=bool(int(os.environ.get("KERNEL_TRACE", "0"))),
    )
    LAST_RESULTS = res

    out = np.empty((B, T, D), np.float32)
    for b in range(B):
        acc = np.asarray(res.results[2 * b]["outT"], np.float32) + np.asarray(
            res.results[2 * b + 1]["outT"], np.float32
        )
        out[b] = acc.T
    return out


# revision 36
# speedup vs baseline: 1.0015x; 1.0015x over previous
"""GQA (n_group == n_head) causal attention kernel for 8 Trainium2 NeuronCores.

Sharding: core c -> (batch b = c//2, head-half hh = c%2).  Each core computes
Q/K/V projections for its 8 heads over the full sequence, causal attention,
and a partial output projection against its 512 rows of Wo.  The host sums
the two partial outputs per batch (the tensor-parallel reduce) and
transposes back.

Device pipeline (per core), all attention operands bf16:
  QT/KT = (x @ W).T          [dout, t]   f32r matmuls, bias added in the
                                         PSUM->SBUF copy (DVE)
  V     = x @ Wv             [t, dout]   column 64 of each V tile is 1.0 so
                                         the PV matmul also accumulates the
                                         softmax denominator
  scT   = K_h @ Q_h.T        [k, q]      both heads of a pair into one
                                         2-bank PSUM tile
  expT  = exp(scT/8)                     one fused Activation per key tile
  pv    = expT.T @ [V_h | 1] [q, 65]     transposed PV: 65-column matmuls
                                         instead of 512-column ones
  ao    = pv[:, :64] / pv[:, 64]         DVE normalize into [q, hd] layout
  aoT   = transpose(ao)      [hd, q]     PE transpose via identity
  outT  = Wo_h.T @ aoT + bo  [dout, q]   partial; host adds core pairs

The attention inner loop is Activation-engine bound (exp), so projection /
out-projection / transpose work is interleaved into the attention tile
stream ("fillers") to keep the tensor engine from idling, paced by a static
cost model of both engines.
"""

import os
import collections
from collections import deque
from contextlib import ExitStack

import numpy as np

import concourse.bass as bass
import concourse.mybir as mybir
import concourse.tile as tile
from concourse import bacc
from concourse.bass import ds, ts
from concourse.bass_utils import run_bass_kernel_spmd

B, T, D = 4, 2048, 1024
H, HD = 16, 64
NCORES = 8
HH = H // 2            # heads per core = 8
DH = HH * HD           # head dims per core = 512
QC = 512               # query block (attention outer tile)
NQC = T // QC          # 4 query blocks
KT = 128               # key tile
TB = 512               # token block for projections
F32 = mybir.dt.float32
F32R = mybir.dt.float32r
BF16 = mybir.dt.bfloat16

# static engine cost estimates (ns) used only to pace filler emission
PE_NS = 1e9 / 2.4e9
ACT_NS = 1e9 / 1.2e9
ACT_OH = float(os.environ.get("KERNEL_ACT_OH", "290"))
THRESH = float(os.environ.get("KERNEL_THRESH", "650"))
TXP_INLINE = os.environ.get("KERNEL_TXP_INLINE", "0") == "1"

LAST_RESULTS = None


def _build_nc():
    nc = bacc.Bacc(
        "TRN2",
        target_bir_lowering=False,
        debug=False,
        enable_asserts=False,
        num_devices=NCORES,
    )

    xT = nc.dram_tensor("xT", [D, T], BF16, kind="ExternalInput").ap()
    wq = nc.dram_tensor("wq", [D, DH], BF16, kind="ExternalInput").ap()
    wk = nc.dram_tensor("wk", [D, DH], BF16, kind="ExternalInput").ap()
    wv = nc.dram_tensor("wv", [D, DH], BF16, kind="ExternalInput").ap()
    wo = nc.dram_tensor("wo", [DH, D], BF16, kind="ExternalInput").ap()
    bq_t = nc.dram_tensor("bq_t", [128, DH // 128], F32, kind="ExternalInput").ap()
    bk_t = nc.dram_tensor("bk_t", [128, DH // 128], F32, kind="ExternalInput").ap()
    bo_t = nc.dram_tensor("bo_t", [128, D // 128], F32, kind="ExternalInput").ap()
    tri = nc.dram_tensor("tri", [128, 128], BF16, kind="ExternalInput").ap()
    ident = nc.dram_tensor("ident", [128, 128], BF16, kind="ExternalInput").ap()
    outT = nc.dram_tensor("outT", [D, T], BF16, kind="ExternalOutput").ap()
    dbg = os.environ.get("KERNEL_DEBUG", "0") == "1"
    if dbg:
        qt_d = nc.dram_tensor("qt_d", [128, 4 * T], BF16, kind="ExternalOutput").ap()
        kt_d = nc.dram_tensor("kt_d", [128, 4 * T], BF16, kind="ExternalOutput").ap()
        v_d = nc.dram_tensor("v_d", [128, (T // KT) * HH * (HD + 1)], BF16, kind="ExternalOutput").ap()
        aoT_d = nc.dram_tensor("aoT_d", [128, 4 * T], BF16, kind="ExternalOutput").ap()

    with tile.TileContext(nc) as tc, ExitStack() as ctx:
        res = ctx.enter_context(tc.tile_pool(name="res", bufs=1))
        # resident SBUF tensors; row c*128+p of qt/kt = local dout
        qt_sb = res.tile([128, 4, T], BF16, tag="qt")
        kt_sb = res.tile([128, 4, T], BF16, tag="kt")
        v_sb = res.tile([128, T // KT, HH, HD + 1], BF16, tag="v")
        aoT_sb = res.tile([128, 4, T], BF16, tag="aoT")
        wq_sb = res.tile([128, 8, DH], BF16, tag="wq")
        wk_sb = res.tile([128, 8, DH], BF16, tag="wk")
        wv_sb = res.tile([128, 8, DH], BF16, tag="wv")
        wo_sb = res.tile([128, 4, D], BF16, tag="wo")
        tri_sb = res.tile([128, 128], BF16, tag="tri")
        id_sb = res.tile([128, 128], BF16, tag="id")
        bq_sb = res.tile([128, 4], F32, tag="bq")
        bk_sb = res.tile([128, 4], F32, tag="bk")
        bo_sb = res.tile([128, 8], F32, tag="bo")

        # PSUM: sc2 2 banks x2 + pv 2 banks x1 + mm 1 bank x2 = 8 banks
        sc2p = ctx.enter_context(tc.tile_pool(name="sc2", bufs=2, space="PSUM"))
        pvp = ctx.enter_context(tc.tile_pool(name="pvp", bufs=1, space="PSUM"))
        mm = ctx.enter_context(tc.tile_pool(name="mm", bufs=2, space="PSUM"))
        xpool = ctx.enter_context(tc.tile_pool(name="xp", bufs=2))
        # et is a ring holding every exp tile of the current (qc, hp) block:
        # PV accumulation groups must run back-to-back per PSUM bank, so PV
        # for chunk cc is deferred until its diagonal tile and then reads all
        # earlier exp tiles.  16 live tiles at qc=3 + slack for the next hp.
        etp = ctx.enter_context(tc.tile_pool(name="et", bufs=18))
        pvr = ctx.enter_context(tc.tile_pool(name="pvr", bufs=2))
        rdp = ctx.enter_context(tc.tile_pool(name="rd", bufs=2))
        stp = ctx.enter_context(tc.tile_pool(name="st", bufs=4))
        # ao tiles live only from normalize until the (qc, hp) transpose
        aop = ctx.enter_context(tc.tile_pool(name="aop", bufs=8))

        # initial DMAs: the SP queue needs ~650ns PER dma_start issue (the
        # transfers are only ~364ns), so batch chunks in pairs and issue on
        # two queues in parallel (SP: wq,wk,biases; ACT: x0,wv,tri,id)
        xts = {}
        xTv = xT.rearrange("(c p) t -> p c t", p=128)
        wqv = wq.rearrange("(c p) d -> p c d", p=128)
        wkv = wk.rearrange("(c p) d -> p c d", p=128)
        wvv = wv.rearrange("(c p) d -> p c d", p=128)

        def load_x(tb):
            xt = xpool.tile([128, 8, TB], BF16, tag="xt", name="xt")
            for c4 in range(2):
                nc.sync.dma_start(
                    out=xt[:, 4 * c4 : 4 * c4 + 4, :],
                    in_=xTv[:, 4 * c4 : 4 * c4 + 4, ts(tb, TB)],
                )
            xts[tb] = xt

        xt0 = xpool.tile([128, 8, TB], BF16, tag="xt", name="xt")
        for c2 in range(4):
            sl2 = slice(2 * c2, 2 * c2 + 2)
            nc.sync.dma_start(out=wq_sb[:, sl2, :], in_=wqv[:, sl2, :])
            nc.scalar.dma_start(out=xt0[:, sl2, :], in_=xTv[:, sl2, ts(0, TB)])
        xts[0] = xt0
        for c2 in range(4):
            sl2 = slice(2 * c2, 2 * c2 + 2)
            nc.sync.dma_start(out=wk_sb[:, sl2, :], in_=wkv[:, sl2, :])
            nc.scalar.dma_start(out=wv_sb[:, sl2, :], in_=wvv[:, sl2, :])
        nc.sync.dma_start(out=bq_sb, in_=bq_t)
        nc.sync.dma_start(out=bk_sb, in_=bk_t)
        nc.sync.dma_start(out=bo_sb, in_=bo_t)
        nc.scalar.dma_start(out=tri_sb, in_=tri)
        nc.scalar.dma_start(out=id_sb, in_=ident)
        nc.vector.memset(v_sb[:, :, :, HD : HD + 1], 1.0)
        wo_loaded = [False]

        def load_wo():
            if not wo_loaded[0]:
                wo_loaded[0] = True
                for c in range(4):
                    nc.sync.dma_start(out=wo_sb[:, c, :], in_=wo[ts(c, 128), :])

        # ---- filler machinery: projq units must land before the next query
        # block; lateq units (out-proj, transposes) have no deadline and are
        # saved for the exp-bound late blocks.
        qq = deque()      # Q units: due before the next query block starts
        projq = deque()   # K/V units: due before the next block's diagonal
        lateq = deque()
        debt = [0.0]  # accumulated ACT-over-PE time not yet filled

        stats = collections.Counter()
        where = ["init"]
        due_count = [0]  # kv units at the head of projq due this block

        def fillers(thresh=THRESH):
            while debt[0] > thresh:
                if qq:
                    qq.popleft()()
                    stats[(where[0], "qq")] += 1
                elif projq:
                    projq.popleft()()
                    if due_count[0] > 0:
                        due_count[0] -= 1
                    stats[(where[0], "kv")] += 1
                elif lateq:
                    lateq.popleft()()
                    stats[(where[0], "late")] += 1
                else:
                    stats[(where[0], "DRY")] += 1
                    break

        def qk_unit(tb, dt, wsb, bias, dst):
            def emit():
                ps = mm.tile([128, TB], F32, tag="mm", name="ps")
                for c in range(8):
                    nc.tensor.matmul(
                        ps, wsb[:, c, ts(dt, 128)], xts[tb][:, c, :],
                        start=(c == 0), stop=(c == 7),
                    )
                nc.vector.tensor_scalar_add(
                    dst[:, dt, ts(tb, TB)], ps, bias[:, dt : dt + 1]
                )
                debt[0] -= 8 * TB * PE_NS
            return emit

        def v_unit(tb, tt):
            def emit():
                ps = mm.tile([128, DH], F32, tag="mm", name="ps")
                for c in range(8):
                    nc.tensor.matmul(
                        ps, xts[tb][:, c, ts(tt, 128)], wv_sb[:, c, :],
                        start=(c == 0), stop=(c == 7),
                    )
                nc.vector.tensor_copy(
                    v_sb[:, tb * (TB // 128) + tt, :, 0:HD],
                    ps.rearrange("p (h c) -> p h c", h=HH),
                )
                debt[0] -= 8 * DH * PE_NS
            return emit

        def q_units(tb):
            return [qk_unit(tb, dt, wq_sb, bq_sb, qt_sb) for dt in range(4)]

        def kv_units(tb):
            return [qk_unit(tb, dt, wk_sb, bk_sb, kt_sb) for dt in range(4)] + [
                v_unit(tb, tt) for tt in range(4)
            ]

        def txp_unit(qc, hp, aot):
            def emit():
                tx = mm.tile([128, 4, 128], BF16, tag="mm", name="tx")
                for cc in range(4):
                    nc.tensor.transpose(tx[:, cc, :], aot[:, cc, :], id_sb)
                nc.vector.tensor_copy(
                    aoT_sb[:, hp, ts(qc, QC)], tx.rearrange("p a b -> p (a b)")
                )
                debt[0] -= 4 * 128 * PE_NS
            return emit

        def oproj_unit(qc, dt):
            def emit():
                if qc == 3 and dt % 2 == 1:
                    ps = sc2p.tile([128, 2, QC], F32, tag="sc", name="ps")[:, 0, :]
                else:
                    ps = mm.tile([128, QC], F32, tag="mm", name="ps")
                for hp in range(4):
                    nc.tensor.matmul(ps, wo_sb[:, hp, ts(dt, 128)],
                                     aoT_sb[:, hp, ts(qc, QC)],
                                     start=(hp == 0), stop=(hp == 3))
                st = stp.tile([128, QC], BF16, tag="st", name="st")
                if qc == 3:
                    # tail: the Activation engine is idle once attention is
                    # done; Identity supports a per-partition bias AP
                    nc.scalar.activation(
                        st, ps, mybir.ActivationFunctionType.Identity,
                        bias=bo_sb[:, dt : dt + 1],
                    )
                else:
                    nc.vector.tensor_scalar_add(st, ps, bo_sb[:, dt : dt + 1])
                nc.sync.dma_start(out=outT[ts(dt, 128), ts(qc, QC)], in_=st)
                debt[0] -= 4 * QC * PE_NS
            return emit

        # token block 0 projections, chunk-major across four concurrent
        # PSUM groups (2 in mm + 2 in sc2) so the tensor engine keeps pace
        # with the streaming weight/x DMAs instead of stalling per group
        def proj_tb0():
            xt = xts[0]
            for kind in ("q", "k", "v"):
                pa = mm.tile([128, TB], F32, tag="mm", name="pa")
                pb = mm.tile([128, TB], F32, tag="mm", name="pb")
                sc = sc2p.tile([128, 2, QC], F32, tag="sc", name="sc")
                pss = [pa, pb, sc[:, 0, :], sc[:, 1, :]]
                for c in range(8):
                    for g in range(4):
                        if kind == "q":
                            nc.tensor.matmul(pss[g], wq_sb[:, c, ts(g, 128)],
                                             xt[:, c, :], start=(c == 0), stop=(c == 7))
                        elif kind == "k":
                            nc.tensor.matmul(pss[g], wk_sb[:, c, ts(g, 128)],
                                             xt[:, c, :], start=(c == 0), stop=(c == 7))
                        else:
                            nc.tensor.matmul(pss[g], xt[:, c, ts(g, 128)],
                                             wv_sb[:, c, :], start=(c == 0), stop=(c == 7))
                for g in range(4):
                    if kind == "q":
                        if g % 2 == 0:
                            nc.vector.tensor_scalar_add(qt_sb[:, g, ts(0, TB)],
                                                        pss[g], bq_sb[:, g : g + 1])
                        else:
                            nc.scalar.activation(
                                qt_sb[:, g, ts(0, TB)], pss[g],
                                mybir.ActivationFunctionType.Identity,
                                bias=bq_sb[:, g : g + 1])
                    elif kind == "k":
                        if g % 2 == 0:
                            nc.vector.tensor_scalar_add(kt_sb[:, g, ts(0, TB)],
                                                        pss[g], bk_sb[:, g : g + 1])
                        else:
                            nc.scalar.activation(
                                kt_sb[:, g, ts(0, TB)], pss[g],
                                mybir.ActivationFunctionType.Identity,
                                bias=bk_sb[:, g : g + 1])
                    else:
                        eng = nc.vector if g % 2 == 0 else nc.scalar
                        if g % 2 == 0:
                            nc.vector.tensor_copy(
                                v_sb[:, g, :, 0:HD],
                                pss[g].rearrange("p (h c) -> p h c", h=HH))
                        else:
                            nc.scalar.activation(
                                v_sb[:, g, :, 0:HD],
                                pss[g].rearrange("p (h c) -> p h c", h=HH),
                                mybir.ActivationFunctionType.Copy)

        proj_tb0()
        debt[0] = 0.0

        for qc in range(NQC):
            due_count[0] = len(projq)  # leftovers: due at this diagonal
            if qc + 1 < NQC:
                load_x(qc + 1)
                qq.extend(q_units(qc + 1))
                projq.extend(kv_units(qc + 1))
            nkt = (qc + 1) * (QC // KT)
            for hp in range(4):
                where[0] = f"qc{qc}hp{hp}"
                # chunk stride padded to 128 floats so each 65-float accumulation
                # region stays inside one 2 KiB PSUM bank (head i -> bank i)
                pv = pvp.tile([128, 2, 4, 128], F32, tag="pv", name="pv")
                ets = []
                for kt in range(nkt):
                    j = kt - qc * (QC // KT)  # >= 0 on the diagonal block
                    q0 = j * KT if j > 0 else 0
                    qn = QC - q0
                    if j == 0 and hp == 0 and due_count[0] > 0:
                        # this block's K/V must be emitted before the first
                        # diagonal QK reads them (engines run in order)
                        while due_count[0] > 0:
                            projq.popleft()()
                            due_count[0] -= 1
                            stats[(where[0], "kv-forced")] += 1
                        debt[0] = min(debt[0], 0.0)
                    sc = sc2p.tile([128, 2, QC], F32, tag="sc", name="sc")
                    for i in range(2):
                        po = i * 64
                        nc.tensor.matmul(
                            sc[:, i, q0:],
                            kt_sb[po : po + 64, hp, ts(kt, 128)],
                            qt_sb[po : po + 64, hp, ds(qc * QC + q0, qn)],
                            start=True,
                            stop=True,
                        )
                    et = etp.tile([128, 2, QC], BF16, tag="et", name="et")
                    nc.scalar.activation(
                        et[:, :, q0:],
                        sc[:, :, q0:],
                        mybir.ActivationFunctionType.Exp,
                        scale=0.125,
                    )
                    ets.append(et)
                    pe_tile = 2 * qn * PE_NS
                    if j >= 0:
                        # causal boundary lies inside q-chunk j only; the
                        # mask is all-ones for chunks right of it
                        for i in range(2):
                            nc.vector.tensor_mul(
                                et[:, i, ts(j, KT)], et[:, i, ts(j, KT)], tri_sb
                            )
                        # chunk j's keys are complete: emit its whole PV
                        # accumulation group back-to-back (one per bank)
                        cc = j
                        for i in range(2):
                            for kk in range(kt + 1):
                                nc.tensor.matmul(
                                    pv[:, i, cc, 0 : HD + 1],
                                    ets[kk][:, i, ts(cc, 128)],
                                    v_sb[:, kk, hp * 2 + i, :],
                                    start=(kk == 0),
                                    stop=(kk == kt),
                                )
                        pe_tile += 2 * (kt + 1) * 65 * PE_NS
                    debt[0] += (2 * qn * ACT_NS + ACT_OH) - pe_tile
                    fillers()
                    if qc == 3 and hp >= 2 and kt % (4 - hp) == 0 and not (
                        qq or projq
                    ) and lateq:
                        # the static debt model under-pops here; these
                        # stretches are exp-bound with plenty queued
                        lateq.popleft()()
                        stats[(where[0], "late-forced")] += 1
                # drain pv quickly so the single PSUM slot frees: exact
                # denominators from PSUM, raw bf16 copy, then normalize
                rd = rdp.tile([128, 2, 4], F32, tag="rd", name="rd")
                nc.vector.reciprocal(rd, pv[:, :, :, HD : HD + 1])
                praw = pvr.tile([128, 2, 4, HD], BF16, tag="praw", name="praw")
                nc.vector.tensor_copy(praw, pv[:, :, :, 0:HD])
                aot = aop.tile([128, 4, 128], BF16, tag="ao", name="aot")
                for i in range(2):
                    for cc in range(4):
                        nc.vector.tensor_scalar_mul(
                            aot[:, cc, i * 64 : (i + 1) * 64],
                            praw[:, i, cc, :],
                            rd[:, i, cc : cc + 1],
                        )
                if TXP_INLINE:
                    txp_unit(qc, hp, aot)()
                else:
                    lateq.append(txp_unit(qc, hp, aot))
            # Q of the next block must be in before it starts; K/V only
            # before its diagonal tiles, so they keep filling the next block
            where[0] = f"qc{qc}end"
            while qq:
                qq.popleft()()
                stats[(where[0], "qq-forced")] += 1
            debt[0] = min(debt[0], 0.0)
            load_wo()
            for dt in range(8):
                lateq.append(oproj_unit(qc, dt))
        where[0] = "tail"
        while lateq:
            lateq.popleft()()
            stats[("tail", "late")] += 1
        if os.environ.get("KERNEL_STATS", "0") == "1":
            for k in sorted(stats):
                print(f"  {k}: {stats[k]}")
        if dbg:
            nc.sync.dma_start(out=qt_d, in_=qt_sb.rearrange("p a b -> p (a b)"))
            nc.sync.dma_start(out=kt_d, in_=kt_sb.rearrange("p a b -> p (a b)"))
            nc.sync.dma_start(out=v_d, in_=v_sb.rearrange("p a b c -> p (a b c)"))
            nc.sync.dma_start(out=aoT_d, in_=aoT_sb.rearrange("p a b -> p (a b)"))

    nc.compile()
    return nc


def kernel(x, Wq, bq, Wk, bk, Wv, bv, Wo, bo):
    global LAST_RESULTS
    import ml_dtypes

    x = np.asarray(x, np.float32)
    Wq, bq = np.asarray(Wq, np.float32), np.asarray(bq, np.float32)
    Wk, bk = np.asarray(Wk, np.float32), np.asarray(bk, np.float32)
    Wv, bv = np.asarray(Wv, np.float32), np.asarray(bv, np.float32)
    Wo, bo = np.asarray(Wo, np.float32), np.asarray(bo, np.float32)

    tri = np.triu(np.ones((128, 128), np.float32)).astype(ml_dtypes.bfloat16)
    ident = np.eye(128, dtype=np.float32).astype(ml_dtypes.bfloat16)

    in_maps = []
    for c in range(NCORES):
        b, hh = c // 2, c % 2
        sl = slice(hh * DH, (hh + 1) * DH)
        # attn out includes +bv per head dim (softmax weights sum to 1), so
        # bv contributes bv_slice @ Wo_slice to this core's partial output;
        # bo itself is carried by the hh == 0 core of each pair.
        bo_eff = bv[sl] @ Wo[sl, :] + (bo if hh == 0 else 0.0)
        in_maps.append(
            {
                "xT": np.ascontiguousarray(x[b].T).astype(ml_dtypes.bfloat16),
                "wq": np.ascontiguousarray(Wq[:, sl]).astype(ml_dtypes.bfloat16),
                "wk": np.ascontiguousarray(Wk[:, sl]).astype(ml_dtypes.bfloat16),
                "wv": np.ascontiguousarray(Wv[:, sl]).astype(ml_dtypes.bfloat16),
                "wo": np.ascontiguousarray(Wo[sl, :]).astype(ml_dtypes.bfloat16),
                "bq_t": np.ascontiguousarray(bq[sl].reshape(4, 128).T),
                "bk_t": np.ascontiguousarray(bk[sl].reshape(4, 128).T),
                "bo_t": np.ascontiguousarray(bo_eff.reshape(8, 128).T),
                "tri": tri,
                "ident": ident,
            }
        )

    nc = _build_nc()
    res = run_bass_kernel_spmd(
        nc,
        in_maps,
        core_ids=list(range(NCORES)),
        trace=bool(int(os.environ.get("KERNEL_TRACE", "0"))),
    )
    LAST_RESULTS = res

    out = np.empty((B, T, D), np.float32)
    for b in range(B):
        acc = np.asarray(res.results[2 * b]["outT"], np.float32) + np.asarray(
            res.results[2 * b + 1]["outT"], np.float32
        )
        out[b] = acc.T
    return out


# revision 37
# speedup vs baseline: 1.0023x; 1.0008x over previous
"""GQA (n_group == n_head) causal attention kernel for 8 Trainium2 NeuronCores.

Sharding: core c -> (batch b = c//2, head-half hh = c%2).  Each core computes
Q/K/V projections for its 8 heads over the full sequence, causal attention,
and a partial output projection against its 512 rows of Wo.  The host sums
the two partial outputs per batch (the tensor-parallel reduce) and
transposes back.

Device pipeline (per core), all attention operands bf16:
  QT/KT = (x @ W).T          [dout, t]   f32r matmuls, bias added in the
                                         PSUM->SBUF copy (DVE)
  V     = x @ Wv             [t, dout]   column 64 of each V tile is 1.0 so
                                         the PV matmul also accumulates the
                                         softmax denominator
  scT   = K_h @ Q_h.T        [k, q]      both heads of a pair into one
                                         2-bank PSUM tile
  expT  = exp(scT/8)                     one fused Activation per key tile
  pv    = expT.T @ [V_h | 1] [q, 65]     transposed PV: 65-column matmuls
                                         instead of 512-column ones
  ao    = pv[:, :64] / pv[:, 64]         DVE normalize into [q, hd] layout
  aoT   = transpose(ao)      [hd, q]     PE transpose via identity
  outT  = Wo_h.T @ aoT + bo  [dout, q]   partial; host adds core pairs

The attention inner loop is Activation-engine bound (exp), so projection /
out-projection / transpose work is interleaved into the attention tile
stream ("fillers") to keep the tensor engine from idling, paced by a static
cost model of both engines.
"""

import os
import collections
from collections import deque
from contextlib import ExitStack

import numpy as np

import concourse.bass as bass
import concourse.mybir as mybir
import concourse.tile as tile
from concourse import bacc
from concourse.bass import ds, ts
from concourse.bass_utils import run_bass_kernel_spmd

B, T, D = 4, 2048, 1024
H, HD = 16, 64
NCORES = 8
HH = H // 2            # heads per core = 8
DH = HH * HD           # head dims per core = 512
QC = 512               # query block (attention outer tile)
NQC = T // QC          # 4 query blocks
KT = 128               # key tile
TB = 512               # token block for projections
F32 = mybir.dt.float32
F32R = mybir.dt.float32r
BF16 = mybir.dt.bfloat16

# static engine cost estimates (ns) used only to pace filler emission
PE_NS = 1e9 / 2.4e9
ACT_NS = 1e9 / 1.2e9
ACT_OH = float(os.environ.get("KERNEL_ACT_OH", "290"))
THRESH = float(os.environ.get("KERNEL_THRESH", "300"))
TXP_INLINE = os.environ.get("KERNEL_TXP_INLINE", "0") == "1"

LAST_RESULTS = None


def _build_nc():
    nc = bacc.Bacc(
        "TRN2",
        target_bir_lowering=False,
        debug=False,
        enable_asserts=False,
        num_devices=NCORES,
    )

    xT = nc.dram_tensor("xT", [D, T], BF16, kind="ExternalInput").ap()
    wq = nc.dram_tensor("wq", [D, DH], BF16, kind="ExternalInput").ap()
    wk = nc.dram_tensor("wk", [D, DH], BF16, kind="ExternalInput").ap()
    wv = nc.dram_tensor("wv", [D, DH], BF16, kind="ExternalInput").ap()
    wo = nc.dram_tensor("wo", [DH, D], BF16, kind="ExternalInput").ap()
    bq_t = nc.dram_tensor("bq_t", [128, DH // 128], F32, kind="ExternalInput").ap()
    bk_t = nc.dram_tensor("bk_t", [128, DH // 128], F32, kind="ExternalInput").ap()
    bo_t = nc.dram_tensor("bo_t", [128, D // 128], F32, kind="ExternalInput").ap()
    tri = nc.dram_tensor("tri", [128, 128], BF16, kind="ExternalInput").ap()
    ident = nc.dram_tensor("ident", [128, 128], BF16, kind="ExternalInput").ap()
    outT = nc.dram_tensor("outT", [D, T], BF16, kind="ExternalOutput").ap()
    dbg = os.environ.get("KERNEL_DEBUG", "0") == "1"
    if dbg:
        qt_d = nc.dram_tensor("qt_d", [128, 4 * T], BF16, kind="ExternalOutput").ap()
        kt_d = nc.dram_tensor("kt_d", [128, 4 * T], BF16, kind="ExternalOutput").ap()
        v_d = nc.dram_tensor("v_d", [128, (T // KT) * HH * (HD + 1)], BF16, kind="ExternalOutput").ap()
        aoT_d = nc.dram_tensor("aoT_d", [128, 4 * T], BF16, kind="ExternalOutput").ap()

    with tile.TileContext(nc) as tc, ExitStack() as ctx:
        res = ctx.enter_context(tc.tile_pool(name="res", bufs=1))
        # resident SBUF tensors; row c*128+p of qt/kt = local dout
        qt_sb = res.tile([128, 4, T], BF16, tag="qt")
        kt_sb = res.tile([128, 4, T], BF16, tag="kt")
        v_sb = res.tile([128, T // KT, HH, HD + 1], BF16, tag="v")
        aoT_sb = res.tile([128, 4, T], BF16, tag="aoT")
        wq_sb = res.tile([128, 8, DH], BF16, tag="wq")
        wk_sb = res.tile([128, 8, DH], BF16, tag="wk")
        wv_sb = res.tile([128, 8, DH], BF16, tag="wv")
        wo_sb = res.tile([128, 4, D], BF16, tag="wo")
        tri_sb = res.tile([128, 128], BF16, tag="tri")
        id_sb = res.tile([128, 128], BF16, tag="id")
        bq_sb = res.tile([128, 4], F32, tag="bq")
        bk_sb = res.tile([128, 4], F32, tag="bk")
        bo_sb = res.tile([128, 8], F32, tag="bo")

        # PSUM: sc2 2 banks x2 + pv 2 banks x1 + mm 1 bank x2 = 8 banks
        sc2p = ctx.enter_context(tc.tile_pool(name="sc2", bufs=2, space="PSUM"))
        pvp = ctx.enter_context(tc.tile_pool(name="pvp", bufs=1, space="PSUM"))
        mm = ctx.enter_context(tc.tile_pool(name="mm", bufs=2, space="PSUM"))
        xpool = ctx.enter_context(tc.tile_pool(name="xp", bufs=2))
        # et is a ring holding every exp tile of the current (qc, hp) block:
        # PV accumulation groups must run back-to-back per PSUM bank, so PV
        # for chunk cc is deferred until its diagonal tile and then reads all
        # earlier exp tiles.  16 live tiles at qc=3 + slack for the next hp.
        etp = ctx.enter_context(tc.tile_pool(name="et", bufs=18))
        pvr = ctx.enter_context(tc.tile_pool(name="pvr", bufs=2))
        rdp = ctx.enter_context(tc.tile_pool(name="rd", bufs=2))
        stp = ctx.enter_context(tc.tile_pool(name="st", bufs=4))
        # ao tiles live only from normalize until the (qc, hp) transpose
        aop = ctx.enter_context(tc.tile_pool(name="aop", bufs=8))

        # initial DMAs: the SP queue needs ~650ns PER dma_start issue (the
        # transfers are only ~364ns), so batch chunks in pairs and issue on
        # two queues in parallel (SP: wq,wk,biases; ACT: x0,wv,tri,id)
        xts = {}
        xTv = xT.rearrange("(c p) t -> p c t", p=128)
        wqv = wq.rearrange("(c p) d -> p c d", p=128)
        wkv = wk.rearrange("(c p) d -> p c d", p=128)
        wvv = wv.rearrange("(c p) d -> p c d", p=128)

        def load_x(tb):
            xt = xpool.tile([128, 8, TB], BF16, tag="xt", name="xt")
            for c4 in range(2):
                nc.sync.dma_start(
                    out=xt[:, 4 * c4 : 4 * c4 + 4, :],
                    in_=xTv[:, 4 * c4 : 4 * c4 + 4, ts(tb, TB)],
                )
            xts[tb] = xt

        xt0 = xpool.tile([128, 8, TB], BF16, tag="xt", name="xt")
        for c2 in range(4):
            sl2 = slice(2 * c2, 2 * c2 + 2)
            nc.sync.dma_start(out=wq_sb[:, sl2, :], in_=wqv[:, sl2, :])
            nc.scalar.dma_start(out=xt0[:, sl2, :], in_=xTv[:, sl2, ts(0, TB)])
        xts[0] = xt0
        for c2 in range(4):
            sl2 = slice(2 * c2, 2 * c2 + 2)
            nc.sync.dma_start(out=wk_sb[:, sl2, :], in_=wkv[:, sl2, :])
            nc.scalar.dma_start(out=wv_sb[:, sl2, :], in_=wvv[:, sl2, :])
        nc.sync.dma_start(out=bq_sb, in_=bq_t)
        nc.sync.dma_start(out=bk_sb, in_=bk_t)
        nc.sync.dma_start(out=bo_sb, in_=bo_t)
        nc.scalar.dma_start(out=tri_sb, in_=tri)
        nc.scalar.dma_start(out=id_sb, in_=ident)
        nc.vector.memset(v_sb[:, :, :, HD : HD + 1], 1.0)
        wo_loaded = [False]

        def load_wo():
            if not wo_loaded[0]:
                wo_loaded[0] = True
                for c in range(4):
                    nc.sync.dma_start(out=wo_sb[:, c, :], in_=wo[ts(c, 128), :])

        # ---- filler machinery: projq units must land before the next query
        # block; lateq units (out-proj, transposes) have no deadline and are
        # saved for the exp-bound late blocks.
        qq = deque()      # Q units: due before the next query block starts
        projq = deque()   # K/V units: due before the next block's diagonal
        lateq = deque()
        debt = [0.0]  # accumulated ACT-over-PE time not yet filled

        stats = collections.Counter()
        where = ["init"]
        due_count = [0]  # kv units at the head of projq due this block

        def fillers(thresh=THRESH):
            while debt[0] > thresh:
                if qq:
                    qq.popleft()()
                    stats[(where[0], "qq")] += 1
                elif projq:
                    projq.popleft()()
                    if due_count[0] > 0:
                        due_count[0] -= 1
                    stats[(where[0], "kv")] += 1
                elif lateq:
                    lateq.popleft()()
                    stats[(where[0], "late")] += 1
                else:
                    stats[(where[0], "DRY")] += 1
                    break

        def qk_unit(tb, dt, wsb, bias, dst):
            def emit():
                ps = mm.tile([128, TB], F32, tag="mm", name="ps")
                for c in range(8):
                    nc.tensor.matmul(
                        ps, wsb[:, c, ts(dt, 128)], xts[tb][:, c, :],
                        start=(c == 0), stop=(c == 7),
                    )
                nc.vector.tensor_scalar_add(
                    dst[:, dt, ts(tb, TB)], ps, bias[:, dt : dt + 1]
                )
                debt[0] -= 8 * TB * PE_NS
            return emit

        def v_unit(tb, tt):
            def emit():
                ps = mm.tile([128, DH], F32, tag="mm", name="ps")
                for c in range(8):
                    nc.tensor.matmul(
                        ps, xts[tb][:, c, ts(tt, 128)], wv_sb[:, c, :],
                        start=(c == 0), stop=(c == 7),
                    )
                nc.vector.tensor_copy(
                    v_sb[:, tb * (TB // 128) + tt, :, 0:HD],
                    ps.rearrange("p (h c) -> p h c", h=HH),
                )
                debt[0] -= 8 * DH * PE_NS
            return emit

        def q_units(tb):
            return [qk_unit(tb, dt, wq_sb, bq_sb, qt_sb) for dt in range(4)]

        def kv_units(tb):
            return [qk_unit(tb, dt, wk_sb, bk_sb, kt_sb) for dt in range(4)] + [
                v_unit(tb, tt) for tt in range(4)
            ]

        def txp_unit(qc, hp, aot):
            def emit():
                tx = mm.tile([128, 4, 128], BF16, tag="mm", name="tx")
                for cc in range(4):
                    nc.tensor.transpose(tx[:, cc, :], aot[:, cc, :], id_sb)
                nc.vector.tensor_copy(
                    aoT_sb[:, hp, ts(qc, QC)], tx.rearrange("p a b -> p (a b)")
                )
                debt[0] -= 4 * 128 * PE_NS
            return emit

        def oproj_unit(qc, dt):
            def emit():
                if qc == 3 and dt % 2 == 1:
                    ps = sc2p.tile([128, 2, QC], F32, tag="sc", name="ps")[:, 0, :]
                else:
                    ps = mm.tile([128, QC], F32, tag="mm", name="ps")
                for hp in range(4):
                    nc.tensor.matmul(ps, wo_sb[:, hp, ts(dt, 128)],
                                     aoT_sb[:, hp, ts(qc, QC)],
                                     start=(hp == 0), stop=(hp == 3))
                st = stp.tile([128, QC], BF16, tag="st", name="st")
                if qc == 3:
                    # tail: the Activation engine is idle once attention is
                    # done; Identity supports a per-partition bias AP
                    nc.scalar.activation(
                        st, ps, mybir.ActivationFunctionType.Identity,
                        bias=bo_sb[:, dt : dt + 1],
                    )
                else:
                    nc.vector.tensor_scalar_add(st, ps, bo_sb[:, dt : dt + 1])
                nc.sync.dma_start(out=outT[ts(dt, 128), ts(qc, QC)], in_=st)
                debt[0] -= 4 * QC * PE_NS
            return emit

        # token block 0 projections, chunk-major across four concurrent
        # PSUM groups (2 in mm + 2 in sc2) so the tensor engine keeps pace
        # with the streaming weight/x DMAs instead of stalling per group
        def proj_tb0():
            xt = xts[0]
            for kind in ("q", "k", "v"):
                pa = mm.tile([128, TB], F32, tag="mm", name="pa")
                pb = mm.tile([128, TB], F32, tag="mm", name="pb")
                sc = sc2p.tile([128, 2, QC], F32, tag="sc", name="sc")
                pss = [pa, pb, sc[:, 0, :], sc[:, 1, :]]
                for c in range(8):
                    for g in range(4):
                        if kind == "q":
                            nc.tensor.matmul(pss[g], wq_sb[:, c, ts(g, 128)],
                                             xt[:, c, :], start=(c == 0), stop=(c == 7))
                        elif kind == "k":
                            nc.tensor.matmul(pss[g], wk_sb[:, c, ts(g, 128)],
                                             xt[:, c, :], start=(c == 0), stop=(c == 7))
                        else:
                            nc.tensor.matmul(pss[g], xt[:, c, ts(g, 128)],
                                             wv_sb[:, c, :], start=(c == 0), stop=(c == 7))
                for g in range(4):
                    if kind == "q":
                        if g % 2 == 0:
                            nc.vector.tensor_scalar_add(qt_sb[:, g, ts(0, TB)],
                                                        pss[g], bq_sb[:, g : g + 1])
                        else:
                            nc.scalar.activation(
                                qt_sb[:, g, ts(0, TB)], pss[g],
                                mybir.ActivationFunctionType.Identity,
                                bias=bq_sb[:, g : g + 1])
                    elif kind == "k":
                        if g % 2 == 0:
                            nc.vector.tensor_scalar_add(kt_sb[:, g, ts(0, TB)],
                                                        pss[g], bk_sb[:, g : g + 1])
                        else:
                            nc.scalar.activation(
                                kt_sb[:, g, ts(0, TB)], pss[g],
                                mybir.ActivationFunctionType.Identity,
                                bias=bk_sb[:, g : g + 1])
                    else:
                        eng = nc.vector if g % 2 == 0 else nc.scalar
                        if g % 2 == 0:
                            nc.vector.tensor_copy(
                                v_sb[:, g, :, 0:HD],
                                pss[g].rearrange("p (h c) -> p h c", h=HH))
                        else:
                            nc.scalar.activation(
                                v_sb[:, g, :, 0:HD],
                                pss[g].rearrange("p (h c) -> p h c", h=HH),
                                mybir.ActivationFunctionType.Copy)

        proj_tb0()
        debt[0] = 0.0

        for qc in range(NQC):
            due_count[0] = len(projq)  # leftovers: due at this diagonal
            if qc + 1 < NQC:
                load_x(qc + 1)
                qq.extend(q_units(qc + 1))
                projq.extend(kv_units(qc + 1))
            nkt = (qc + 1) * (QC // KT)
            for hp in range(4):
                where[0] = f"qc{qc}hp{hp}"
                # chunk stride padded to 128 floats so each 65-float accumulation
                # region stays inside one 2 KiB PSUM bank (head i -> bank i)
                pv = pvp.tile([128, 2, 4, 128], F32, tag="pv", name="pv")
                ets = []
                for kt in range(nkt):
                    j = kt - qc * (QC // KT)  # >= 0 on the diagonal block
                    q0 = j * KT if j > 0 else 0
                    qn = QC - q0
                    if j == 0 and hp == 0 and due_count[0] > 0:
                        # this block's K/V must be emitted before the first
                        # diagonal QK reads them (engines run in order)
                        while due_count[0] > 0:
                            projq.popleft()()
                            due_count[0] -= 1
                            stats[(where[0], "kv-forced")] += 1
                        debt[0] = min(debt[0], 0.0)
                    sc = sc2p.tile([128, 2, QC], F32, tag="sc", name="sc")
                    for i in range(2):
                        po = i * 64
                        nc.tensor.matmul(
                            sc[:, i, q0:],
                            kt_sb[po : po + 64, hp, ts(kt, 128)],
                            qt_sb[po : po + 64, hp, ds(qc * QC + q0, qn)],
                            start=True,
                            stop=True,
                        )
                    et = etp.tile([128, 2, QC], BF16, tag="et", name="et")
                    nc.scalar.activation(
                        et[:, :, q0:],
                        sc[:, :, q0:],
                        mybir.ActivationFunctionType.Exp,
                        scale=0.125,
                    )
                    ets.append(et)
                    pe_tile = 2 * qn * PE_NS
                    if j >= 0:
                        # causal boundary lies inside q-chunk j only; the
                        # mask is all-ones for chunks right of it
                        for i in range(2):
                            nc.vector.tensor_mul(
                                et[:, i, ts(j, KT)], et[:, i, ts(j, KT)], tri_sb
                            )
                        # chunk j's keys are complete: emit its whole PV
                        # accumulation group back-to-back (one per bank)
                        cc = j
                        for i in range(2):
                            for kk in range(kt + 1):
                                nc.tensor.matmul(
                                    pv[:, i, cc, 0 : HD + 1],
                                    ets[kk][:, i, ts(cc, 128)],
                                    v_sb[:, kk, hp * 2 + i, :],
                                    start=(kk == 0),
                                    stop=(kk == kt),
                                )
                        pe_tile += 2 * (kt + 1) * 65 * PE_NS
                    debt[0] += (2 * qn * ACT_NS + ACT_OH) - pe_tile
                    fillers()
                    if qc == 3 and hp >= 2 and kt % (4 - hp) == 0 and not (
                        qq or projq
                    ) and lateq:
                        # the static debt model under-pops here; these
                        # stretches are exp-bound with plenty queued
                        lateq.popleft()()
                        stats[(where[0], "late-forced")] += 1
                # drain pv quickly so the single PSUM slot frees: exact
                # denominators from PSUM, raw bf16 copy, then normalize
                rd = rdp.tile([128, 2, 4], F32, tag="rd", name="rd")
                nc.vector.reciprocal(rd, pv[:, :, :, HD : HD + 1])
                praw = pvr.tile([128, 2, 4, HD], BF16, tag="praw", name="praw")
                nc.vector.tensor_copy(praw, pv[:, :, :, 0:HD])
                aot = aop.tile([128, 4, 128], BF16, tag="ao", name="aot")
                for i in range(2):
                    for cc in range(4):
                        nc.vector.tensor_scalar_mul(
                            aot[:, cc, i * 64 : (i + 1) * 64],
                            praw[:, i, cc, :],
                            rd[:, i, cc : cc + 1],
                        )
                if TXP_INLINE:
                    txp_unit(qc, hp, aot)()
                else:
                    lateq.append(txp_unit(qc, hp, aot))
            # Q of the next block must be in before it starts; K/V only
            # before its diagonal tiles, so they keep filling the next block
            where[0] = f"qc{qc}end"
            while qq:
                qq.popleft()()
                stats[(where[0], "qq-forced")] += 1
            debt[0] = min(debt[0], 0.0)
            load_wo()
            for dt in range(8):
                lateq.append(oproj_unit(qc, dt))
        where[0] = "tail"
        while lateq:
            lateq.popleft()()
            stats[("tail", "late")] += 1
        if os.environ.get("KERNEL_STATS", "0") == "1":
            for k in sorted(stats):
                print(f"  {k}: {stats[k]}")
        if dbg:
            nc.sync.dma_start(out=qt_d, in_=qt_sb.rearrange("p a b -> p (a b)"))
            nc.sync.dma_start(out=kt_d, in_=kt_sb.rearrange("p a b -> p (a b)"))
            nc.sync.dma_start(out=v_d, in_=v_sb.rearrange("p a b c -> p (a b c)"))
            nc.sync.dma_start(out=aoT_d, in_=aoT_sb.rearrange("p a b -> p (a b)"))

    nc.compile()
    return nc


def kernel(x, Wq, bq, Wk, bk, Wv, bv, Wo, bo):
    global LAST_RESULTS
    import ml_dtypes

    x = np.asarray(x, np.float32)
    Wq, bq = np.asarray(Wq, np.float32), np.asarray(bq, np.float32)
    Wk, bk = np.asarray(Wk, np.float32), np.asarray(bk, np.float32)
    Wv, bv = np.asarray(Wv, np.float32), np.asarray(bv, np.float32)
    Wo, bo = np.asarray(Wo, np.float32), np.asarray(bo, np.float32)

    tri = np.triu(np.ones((128, 128), np.float32)).astype(ml_dtypes.bfloat16)
    ident = np.eye(128, dtype=np.float32).astype(ml_dtypes.bfloat16)

    in_maps = []
    for c in range(NCORES):
        b, hh = c // 2, c % 2
        sl = slice(hh * DH, (hh + 1) * DH)
        # attn out includes +bv per head dim (softmax weights sum to 1), so
        # bv contributes bv_slice @ Wo_slice to this core's partial output;
        # bo itself is carried by the hh == 0 core of each pair.
        bo_eff = bv[sl] @ Wo[sl, :] + (bo if hh == 0 else 0.0)
        in_maps.append(
            {
                "xT": np.ascontiguousarray(x[b].T).astype(ml_dtypes.bfloat16),
                "wq": np.ascontiguousarray(Wq[:, sl]).astype(ml_dtypes.bfloat16),
                "wk": np.ascontiguousarray(Wk[:, sl]).astype(ml_dtypes.bfloat16),
                "wv": np.ascontiguousarray(Wv[:, sl]).astype(ml_dtypes.bfloat16),
                "wo": np.ascontiguousarray(Wo[sl, :]).astype(ml_dtypes.bfloat16),
                "bq_t": np.ascontiguousarray(bq[sl].reshape(4, 128).T),
                "bk_t": np.ascontiguousarray(bk[sl].reshape(4, 128).T),
                "bo_t": np.ascontiguousarray(bo_eff.reshape(8, 128).T),
                "tri": tri,
                "ident": ident,
            }
        )

    nc = _build_nc()
    res = run_bass_kernel_spmd(
        nc,
        in_maps,
        core_ids=list(range(NCORES)),
        trace=bool(int(os.environ.get("KERNEL_TRACE", "0"))),
    )
    LAST_RESULTS = res

    out = np.empty((B, T, D), np.float32)
    for b in range(B):
        acc = np.asarray(res.results[2 * b]["outT"], np.float32) + np.asarray(
            res.results[2 * b + 1]["outT"], np.float32
        )
        out[b] = acc.T
    return out


# revision 38
# speedup vs baseline: 1.0062x; 1.0039x over previous
"""GQA (n_group == n_head) causal attention kernel for 8 Trainium2 NeuronCores.

Sharding: core c -> (batch b = c//2, head-half hh = c%2).  Each core computes
Q/K/V projections for its 8 heads over the full sequence, causal attention,
and a partial output projection against its 512 rows of Wo.  The host sums
the two partial outputs per batch (the tensor-parallel reduce) and
transposes back.

Device pipeline (per core), all attention operands bf16:
  QT/KT = (x @ W).T          [dout, t]   f32r matmuls, bias added in the
                                         PSUM->SBUF copy (DVE)
  V     = x @ Wv             [t, dout]   column 64 of each V tile is 1.0 so
                                         the PV matmul also accumulates the
                                         softmax denominator
  scT   = K_h @ Q_h.T        [k, q]      both heads of a pair into one
                                         2-bank PSUM tile
  expT  = exp(scT/8)                     one fused Activation per key tile
  pv    = expT.T @ [V_h | 1] [q, 65]     transposed PV: 65-column matmuls
                                         instead of 512-column ones
  ao    = pv[:, :64] / pv[:, 64]         DVE normalize into [q, hd] layout
  aoT   = transpose(ao)      [hd, q]     PE transpose via identity
  outT  = Wo_h.T @ aoT + bo  [dout, q]   partial; host adds core pairs

The attention inner loop is Activation-engine bound (exp), so projection /
out-projection / transpose work is interleaved into the attention tile
stream ("fillers") to keep the tensor engine from idling, paced by a static
cost model of both engines.
"""

import os
import collections
from collections import deque
from contextlib import ExitStack

import numpy as np

import concourse.bass as bass
import concourse.mybir as mybir
import concourse.tile as tile
from concourse import bacc
from concourse.bass import ds, ts
from concourse.bass_utils import run_bass_kernel_spmd

B, T, D = 4, 2048, 1024
H, HD = 16, 64
NCORES = 8
HH = H // 2            # heads per core = 8
DH = HH * HD           # head dims per core = 512
QC = 512               # query block (attention outer tile)
NQC = T // QC          # 4 query blocks
KT = 128               # key tile
TB = 512               # token block for projections
F32 = mybir.dt.float32
F32R = mybir.dt.float32r
BF16 = mybir.dt.bfloat16

# static engine cost estimates (ns) used only to pace filler emission
PE_NS = 1e9 / 2.4e9
ACT_NS = 1e9 / 1.2e9
ACT_OH = float(os.environ.get("KERNEL_ACT_OH", "290"))
THRESH = float(os.environ.get("KERNEL_THRESH", "300"))
TXP_INLINE = os.environ.get("KERNEL_TXP_INLINE", "0") == "1"

LAST_RESULTS = None


def _build_nc():
    nc = bacc.Bacc(
        "TRN2",
        target_bir_lowering=False,
        debug=False,
        enable_asserts=False,
        num_devices=NCORES,
    )

    xT = nc.dram_tensor("xT", [D, T], BF16, kind="ExternalInput").ap()
    wq = nc.dram_tensor("wq", [D, DH], BF16, kind="ExternalInput").ap()
    wk = nc.dram_tensor("wk", [D, DH], BF16, kind="ExternalInput").ap()
    wv = nc.dram_tensor("wv", [D, DH], BF16, kind="ExternalInput").ap()
    wo = nc.dram_tensor("wo", [DH, D], BF16, kind="ExternalInput").ap()
    bq_t = nc.dram_tensor("bq_t", [128, DH // 128], F32, kind="ExternalInput").ap()
    bk_t = nc.dram_tensor("bk_t", [128, DH // 128], F32, kind="ExternalInput").ap()
    bo_t = nc.dram_tensor("bo_t", [128, D // 128], F32, kind="ExternalInput").ap()
    tri = nc.dram_tensor("tri", [128, 128], BF16, kind="ExternalInput").ap()
    ident = nc.dram_tensor("ident", [128, 128], BF16, kind="ExternalInput").ap()
    outT = nc.dram_tensor("outT", [D, T], BF16, kind="ExternalOutput").ap()
    dbg = os.environ.get("KERNEL_DEBUG", "0") == "1"
    if dbg:
        qt_d = nc.dram_tensor("qt_d", [128, 4 * T], BF16, kind="ExternalOutput").ap()
        kt_d = nc.dram_tensor("kt_d", [128, 4 * T], BF16, kind="ExternalOutput").ap()
        v_d = nc.dram_tensor("v_d", [128, (T // KT) * HH * (HD + 1)], BF16, kind="ExternalOutput").ap()
        aoT_d = nc.dram_tensor("aoT_d", [128, 4 * T], BF16, kind="ExternalOutput").ap()

    with tile.TileContext(nc) as tc, ExitStack() as ctx:
        res = ctx.enter_context(tc.tile_pool(name="res", bufs=1))
        # resident SBUF tensors; row c*128+p of qt/kt = local dout
        qt_sb = res.tile([128, 4, T], BF16, tag="qt")
        kt_sb = res.tile([128, 4, T], BF16, tag="kt")
        v_sb = res.tile([128, T // KT, HH, HD + 1], BF16, tag="v")
        aoT_sb = res.tile([128, 4, T], BF16, tag="aoT")
        wq_sb = res.tile([128, 8, DH], BF16, tag="wq")
        wk_sb = res.tile([128, 8, DH], BF16, tag="wk")
        wv_sb = res.tile([128, 8, DH], BF16, tag="wv")
        wo_sb = res.tile([128, 4, D], BF16, tag="wo")
        tri_sb = res.tile([128, 128], BF16, tag="tri")
        id_sb = res.tile([128, 128], BF16, tag="id")
        bq_sb = res.tile([128, 4], F32, tag="bq")
        bk_sb = res.tile([128, 4], F32, tag="bk")
        bo_sb = res.tile([128, 8], F32, tag="bo")

        # PSUM: sc2 2 banks x2 + pv 2 banks x1 + mm 1 bank x2 = 8 banks
        sc2p = ctx.enter_context(tc.tile_pool(name="sc2", bufs=2, space="PSUM"))
        pvp = ctx.enter_context(tc.tile_pool(name="pvp", bufs=1, space="PSUM"))
        mm = ctx.enter_context(tc.tile_pool(name="mm", bufs=2, space="PSUM"))
        xpool = ctx.enter_context(tc.tile_pool(name="xp", bufs=2))
        # et is a ring holding every exp tile of the current (qc, hp) block:
        # PV accumulation groups must run back-to-back per PSUM bank, so PV
        # for chunk cc is deferred until its diagonal tile and then reads all
        # earlier exp tiles.  16 live tiles at qc=3 + slack for the next hp.
        etp = ctx.enter_context(tc.tile_pool(name="et", bufs=18))
        pvr = ctx.enter_context(tc.tile_pool(name="pvr", bufs=3))
        rdp = ctx.enter_context(tc.tile_pool(name="rd", bufs=3))
        stp = ctx.enter_context(tc.tile_pool(name="st", bufs=4))
        # ao tiles live only from normalize until the (qc, hp) transpose
        aop = ctx.enter_context(tc.tile_pool(name="aop", bufs=8))

        # initial DMAs: the SP queue needs ~650ns PER dma_start issue (the
        # transfers are only ~364ns), so batch chunks in pairs and issue on
        # two queues in parallel (SP: wq,wk,biases; ACT: x0,wv,tri,id)
        xts = {}
        xTv = xT.rearrange("(c p) t -> p c t", p=128)
        wqv = wq.rearrange("(c p) d -> p c d", p=128)
        wkv = wk.rearrange("(c p) d -> p c d", p=128)
        wvv = wv.rearrange("(c p) d -> p c d", p=128)

        def load_x(tb):
            xt = xpool.tile([128, 8, TB], BF16, tag="xt", name="xt")
            for c4 in range(2):
                nc.sync.dma_start(
                    out=xt[:, 4 * c4 : 4 * c4 + 4, :],
                    in_=xTv[:, 4 * c4 : 4 * c4 + 4, ts(tb, TB)],
                )
            xts[tb] = xt

        xt0 = xpool.tile([128, 8, TB], BF16, tag="xt", name="xt")
        for c2 in range(4):
            sl2 = slice(2 * c2, 2 * c2 + 2)
            nc.sync.dma_start(out=wq_sb[:, sl2, :], in_=wqv[:, sl2, :])
            nc.scalar.dma_start(out=xt0[:, sl2, :], in_=xTv[:, sl2, ts(0, TB)])
        xts[0] = xt0
        for c2 in range(4):
            sl2 = slice(2 * c2, 2 * c2 + 2)
            nc.sync.dma_start(out=wk_sb[:, sl2, :], in_=wkv[:, sl2, :])
            nc.scalar.dma_start(out=wv_sb[:, sl2, :], in_=wvv[:, sl2, :])
        nc.sync.dma_start(out=bq_sb, in_=bq_t)
        nc.sync.dma_start(out=bk_sb, in_=bk_t)
        nc.sync.dma_start(out=bo_sb, in_=bo_t)
        nc.scalar.dma_start(out=tri_sb, in_=tri)
        nc.scalar.dma_start(out=id_sb, in_=ident)
        nc.vector.memset(v_sb[:, :, :, HD : HD + 1], 1.0)
        wo_loaded = [False]

        def load_wo():
            if not wo_loaded[0]:
                wo_loaded[0] = True
                for c in range(4):
                    nc.sync.dma_start(out=wo_sb[:, c, :], in_=wo[ts(c, 128), :])

        # ---- filler machinery: projq units must land before the next query
        # block; lateq units (out-proj, transposes) have no deadline and are
        # saved for the exp-bound late blocks.
        qq = deque()      # Q units: due before the next query block starts
        projq = deque()   # K/V units: due before the next block's diagonal
        lateq = deque()
        debt = [0.0]  # accumulated ACT-over-PE time not yet filled

        stats = collections.Counter()
        where = ["init"]
        due_count = [0]  # kv units at the head of projq due this block

        def fillers(thresh=THRESH):
            while debt[0] > thresh:
                if qq:
                    qq.popleft()()
                    stats[(where[0], "qq")] += 1
                elif projq:
                    projq.popleft()()
                    if due_count[0] > 0:
                        due_count[0] -= 1
                    stats[(where[0], "kv")] += 1
                elif lateq:
                    lateq.popleft()()
                    stats[(where[0], "late")] += 1
                else:
                    stats[(where[0], "DRY")] += 1
                    break

        def qk_unit(tb, dt, wsb, bias, dst):
            def emit():
                ps = mm.tile([128, TB], F32, tag="mm", name="ps")
                for c in range(8):
                    nc.tensor.matmul(
                        ps, wsb[:, c, ts(dt, 128)], xts[tb][:, c, :],
                        start=(c == 0), stop=(c == 7),
                    )
                nc.vector.tensor_scalar_add(
                    dst[:, dt, ts(tb, TB)], ps, bias[:, dt : dt + 1]
                )
                debt[0] -= 8 * TB * PE_NS
            return emit

        def v_unit(tb, tt):
            def emit():
                ps = mm.tile([128, DH], F32, tag="mm", name="ps")
                for c in range(8):
                    nc.tensor.matmul(
                        ps, xts[tb][:, c, ts(tt, 128)], wv_sb[:, c, :],
                        start=(c == 0), stop=(c == 7),
                    )
                nc.vector.tensor_copy(
                    v_sb[:, tb * (TB // 128) + tt, :, 0:HD],
                    ps.rearrange("p (h c) -> p h c", h=HH),
                )
                debt[0] -= 8 * DH * PE_NS
            return emit

        def q_units(tb):
            return [qk_unit(tb, dt, wq_sb, bq_sb, qt_sb) for dt in range(4)]

        def kv_units(tb):
            return [qk_unit(tb, dt, wk_sb, bk_sb, kt_sb) for dt in range(4)] + [
                v_unit(tb, tt) for tt in range(4)
            ]

        def txp_unit(qc, hp, aot):
            def emit():
                tx = mm.tile([128, 4, 128], BF16, tag="mm", name="tx")
                for cc in range(4):
                    nc.tensor.transpose(tx[:, cc, :], aot[:, cc, :], id_sb)
                nc.vector.tensor_copy(
                    aoT_sb[:, hp, ts(qc, QC)], tx.rearrange("p a b -> p (a b)")
                )
                debt[0] -= 4 * 128 * PE_NS
            return emit

        def oproj_unit(qc, dt):
            def emit():
                if qc == 3 and dt % 2 == 1:
                    ps = sc2p.tile([128, 2, QC], F32, tag="sc", name="ps")[:, 0, :]
                else:
                    ps = mm.tile([128, QC], F32, tag="mm", name="ps")
                for hp in range(4):
                    nc.tensor.matmul(ps, wo_sb[:, hp, ts(dt, 128)],
                                     aoT_sb[:, hp, ts(qc, QC)],
                                     start=(hp == 0), stop=(hp == 3))
                st = stp.tile([128, QC], BF16, tag="st", name="st")
                if qc == 3:
                    # tail: the Activation engine is idle once attention is
                    # done; Identity supports a per-partition bias AP
                    nc.scalar.activation(
                        st, ps, mybir.ActivationFunctionType.Identity,
                        bias=bo_sb[:, dt : dt + 1],
                    )
                else:
                    nc.vector.tensor_scalar_add(st, ps, bo_sb[:, dt : dt + 1])
                nc.sync.dma_start(out=outT[ts(dt, 128), ts(qc, QC)], in_=st)
                debt[0] -= 4 * QC * PE_NS
            return emit

        # token block 0 projections, chunk-major across four concurrent
        # PSUM groups (2 in mm + 2 in sc2) so the tensor engine keeps pace
        # with the streaming weight/x DMAs instead of stalling per group
        def proj_tb0():
            xt = xts[0]
            for kind in ("q", "k", "v"):
                pa = mm.tile([128, TB], F32, tag="mm", name="pa")
                pb = mm.tile([128, TB], F32, tag="mm", name="pb")
                sc = sc2p.tile([128, 2, QC], F32, tag="sc", name="sc")
                pss = [pa, pb, sc[:, 0, :], sc[:, 1, :]]
                for c in range(8):
                    for g in range(4):
                        if kind == "q":
                            nc.tensor.matmul(pss[g], wq_sb[:, c, ts(g, 128)],
                                             xt[:, c, :], start=(c == 0), stop=(c == 7))
                        elif kind == "k":
                            nc.tensor.matmul(pss[g], wk_sb[:, c, ts(g, 128)],
                                             xt[:, c, :], start=(c == 0), stop=(c == 7))
                        else:
                            nc.tensor.matmul(pss[g], xt[:, c, ts(g, 128)],
                                             wv_sb[:, c, :], start=(c == 0), stop=(c == 7))
                for g in range(4):
                    if kind == "q":
                        if g % 2 == 0:
                            nc.vector.tensor_scalar_add(qt_sb[:, g, ts(0, TB)],
                                                        pss[g], bq_sb[:, g : g + 1])
                        else:
                            nc.scalar.activation(
                                qt_sb[:, g, ts(0, TB)], pss[g],
                                mybir.ActivationFunctionType.Identity,
                                bias=bq_sb[:, g : g + 1])
                    elif kind == "k":
                        if g % 2 == 0:
                            nc.vector.tensor_scalar_add(kt_sb[:, g, ts(0, TB)],
                                                        pss[g], bk_sb[:, g : g + 1])
                        else:
                            nc.scalar.activation(
                                kt_sb[:, g, ts(0, TB)], pss[g],
                                mybir.ActivationFunctionType.Identity,
                                bias=bk_sb[:, g : g + 1])
                    else:
                        eng = nc.vector if g % 2 == 0 else nc.scalar
                        if g % 2 == 0:
                            nc.vector.tensor_copy(
                                v_sb[:, g, :, 0:HD],
                                pss[g].rearrange("p (h c) -> p h c", h=HH))
                        else:
                            nc.scalar.activation(
                                v_sb[:, g, :, 0:HD],
                                pss[g].rearrange("p (h c) -> p h c", h=HH),
                                mybir.ActivationFunctionType.Copy)

        proj_tb0()
        debt[0] = 0.0

        for qc in range(NQC):
            due_count[0] = len(projq)  # leftovers: due at this diagonal
            if qc + 1 < NQC:
                load_x(qc + 1)
                qq.extend(q_units(qc + 1))
                projq.extend(kv_units(qc + 1))
            nkt = (qc + 1) * (QC // KT)
            for hp in range(4):
                where[0] = f"qc{qc}hp{hp}"
                # chunk stride padded to 128 floats so each 65-float accumulation
                # region stays inside one 2 KiB PSUM bank (head i -> bank i)
                pv = pvp.tile([128, 2, 4, 128], F32, tag="pv", name="pv")
                ets = []
                for kt in range(nkt):
                    j = kt - qc * (QC // KT)  # >= 0 on the diagonal block
                    q0 = j * KT if j > 0 else 0
                    qn = QC - q0
                    if j == 0 and hp == 0 and due_count[0] > 0:
                        # this block's K/V must be emitted before the first
                        # diagonal QK reads them (engines run in order)
                        while due_count[0] > 0:
                            projq.popleft()()
                            due_count[0] -= 1
                            stats[(where[0], "kv-forced")] += 1
                        debt[0] = min(debt[0], 0.0)
                    sc = sc2p.tile([128, 2, QC], F32, tag="sc", name="sc")
                    for i in range(2):
                        po = i * 64
                        nc.tensor.matmul(
                            sc[:, i, q0:],
                            kt_sb[po : po + 64, hp, ts(kt, 128)],
                            qt_sb[po : po + 64, hp, ds(qc * QC + q0, qn)],
                            start=True,
                            stop=True,
                        )
                    et = etp.tile([128, 2, QC], BF16, tag="et", name="et")
                    nc.scalar.activation(
                        et[:, :, q0:],
                        sc[:, :, q0:],
                        mybir.ActivationFunctionType.Exp,
                        scale=0.125,
                    )
                    ets.append(et)
                    pe_tile = 2 * qn * PE_NS
                    if j >= 0:
                        # causal boundary lies inside q-chunk j only; the
                        # mask is all-ones for chunks right of it
                        for i in range(2):
                            nc.vector.tensor_mul(
                                et[:, i, ts(j, KT)], et[:, i, ts(j, KT)], tri_sb
                            )
                        # chunk j's keys are complete: emit its whole PV
                        # accumulation group back-to-back (one per bank)
                        cc = j
                        for i in range(2):
                            for kk in range(kt + 1):
                                nc.tensor.matmul(
                                    pv[:, i, cc, 0 : HD + 1],
                                    ets[kk][:, i, ts(cc, 128)],
                                    v_sb[:, kk, hp * 2 + i, :],
                                    start=(kk == 0),
                                    stop=(kk == kt),
                                )
                        pe_tile += 2 * (kt + 1) * 65 * PE_NS
                    debt[0] += (2 * qn * ACT_NS + ACT_OH) - pe_tile
                    fillers()
                    if qc == 3 and hp >= 2 and kt % (4 - hp) == 0 and not (
                        qq or projq
                    ) and lateq:
                        # the static debt model under-pops here; these
                        # stretches are exp-bound with plenty queued
                        lateq.popleft()()
                        stats[(where[0], "late-forced")] += 1
                # drain pv quickly so the single PSUM slot frees: exact
                # denominators from PSUM, raw bf16 copy, then normalize
                rd = rdp.tile([128, 2, 4], F32, tag="rd", name="rd")
                nc.vector.reciprocal(rd, pv[:, :, :, HD : HD + 1])
                praw = pvr.tile([128, 2, 4, HD], BF16, tag="praw", name="praw")
                nc.vector.tensor_copy(praw, pv[:, :, :, 0:HD])
                aot = aop.tile([128, 4, 128], BF16, tag="ao", name="aot")
                for i in range(2):
                    for cc in range(4):
                        nc.vector.tensor_scalar_mul(
                            aot[:, cc, i * 64 : (i + 1) * 64],
                            praw[:, i, cc, :],
                            rd[:, i, cc : cc + 1],
                        )
                if TXP_INLINE:
                    txp_unit(qc, hp, aot)()
                else:
                    lateq.append(txp_unit(qc, hp, aot))
            # Q of the next block must be in before it starts; K/V only
            # before its diagonal tiles, so they keep filling the next block
            where[0] = f"qc{qc}end"
            while qq:
                qq.popleft()()
                stats[(where[0], "qq-forced")] += 1
            debt[0] = min(debt[0], 0.0)
            load_wo()
            for dt in range(8):
                lateq.append(oproj_unit(qc, dt))
        where[0] = "tail"
        while lateq:
            lateq.popleft()()
            stats[("tail", "late")] += 1
        if os.environ.get("KERNEL_STATS", "0") == "1":
            for k in sorted(stats):
                print(f"  {k}: {stats[k]}")
        if dbg:
            nc.sync.dma_start(out=qt_d, in_=qt_sb.rearrange("p a b -> p (a b)"))
            nc.sync.dma_start(out=kt_d, in_=kt_sb.rearrange("p a b -> p (a b)"))
            nc.sync.dma_start(out=v_d, in_=v_sb.rearrange("p a b c -> p (a b c)"))
            nc.sync.dma_start(out=aoT_d, in_=aoT_sb.rearrange("p a b -> p (a b)"))

    nc.compile()
    return nc


def kernel(x, Wq, bq, Wk, bk, Wv, bv, Wo, bo):
    global LAST_RESULTS
    import ml_dtypes

    x = np.asarray(x, np.float32)
    Wq, bq = np.asarray(Wq, np.float32), np.asarray(bq, np.float32)
    Wk, bk = np.asarray(Wk, np.float32), np.asarray(bk, np.float32)
    Wv, bv = np.asarray(Wv, np.float32), np.asarray(bv, np.float32)
    Wo, bo = np.asarray(Wo, np.float32), np.asarray(bo, np.float32)

    tri = np.triu(np.ones((128, 128), np.float32)).astype(ml_dtypes.bfloat16)
    ident = np.eye(128, dtype=np.float32).astype(ml_dtypes.bfloat16)

    in_maps = []
    for c in range(NCORES):
        b, hh = c // 2, c % 2
        sl = slice(hh * DH, (hh + 1) * DH)
        # attn out includes +bv per head dim (softmax weights sum to 1), so
        # bv contributes bv_slice @ Wo_slice to this core's partial output;
        # bo itself is carried by the hh == 0 core of each pair.
        bo_eff = bv[sl] @ Wo[sl, :] + (bo if hh == 0 else 0.0)
        in_maps.append(
            {
                "xT": np.ascontiguousarray(x[b].T).astype(ml_dtypes.bfloat16),
                "wq": np.ascontiguousarray(Wq[:, sl]).astype(ml_dtypes.bfloat16),
                "wk": np.ascontiguousarray(Wk[:, sl]).astype(ml_dtypes.bfloat16),
                "wv": np.ascontiguousarray(Wv[:, sl]).astype(ml_dtypes.bfloat16),
                "wo": np.ascontiguousarray(Wo[sl, :]).astype(ml_dtypes.bfloat16),
                "bq_t": np.ascontiguousarray(bq[sl].reshape(4, 128).T),
                "bk_t": np.ascontiguousarray(bk[sl].reshape(4, 128).T),
                "bo_t": np.ascontiguousarray(bo_eff.reshape(8, 128).T),
                "tri": tri,
                "ident": ident,
            }
        )

    nc = _build_nc()
    res = run_bass_kernel_spmd(
        nc,
        in_maps,
        core_ids=list(range(NCORES)),
        trace=bool(int(os.environ.get("KERNEL_TRACE", "0"))),
    )
    LAST_RESULTS = res

    out = np.empty((B, T, D), np.float32)
    for b in range(B):
        acc = np.asarray(res.results[2 * b]["outT"], np.float32) + np.asarray(
            res.results[2 * b + 1]["outT"], np.float32
        )
        out[b] = acc.T
    return out


# revision 39
# speedup vs baseline: 1.0091x; 1.0029x over previous
"""GQA (n_group == n_head) causal attention kernel for 8 Trainium2 NeuronCores.

Sharding: core c -> (batch b = c//2, head-half hh = c%2).  Each core computes
Q/K/V projections for its 8 heads over the full sequence, causal attention,
and a partial output projection against its 512 rows of Wo.  The host sums
the two partial outputs per batch (the tensor-parallel reduce) and
transposes back.

Device pipeline (per core), all attention operands bf16:
  QT/KT = (x @ W).T          [dout, t]   f32r matmuls, bias added in the
                                         PSUM->SBUF copy (DVE)
  V     = x @ Wv             [t, dout]   column 64 of each V tile is 1.0 so
                                         the PV matmul also accumulates the
                                         softmax denominator
  scT   = K_h @ Q_h.T        [k, q]      both heads of a pair into one
                                         2-bank PSUM tile
  expT  = exp(scT/8)                     one fused Activation per key tile
  pv    = expT.T @ [V_h | 1] [q, 65]     transposed PV: 65-column matmuls
                                         instead of 512-column ones
  ao    = pv[:, :64] / pv[:, 64]         DVE normalize into [q, hd] layout
  aoT   = transpose(ao)      [hd, q]     PE transpose via identity
  outT  = Wo_h.T @ aoT + bo  [dout, q]   partial; host adds core pairs

The attention inner loop is Activation-engine bound (exp), so projection /
out-projection / transpose work is interleaved into the attention tile
stream ("fillers") to keep the tensor engine from idling, paced by a static
cost model of both engines.
"""

import os
import collections
from collections import deque
from contextlib import ExitStack

import numpy as np

import concourse.bass as bass
import concourse.mybir as mybir
import concourse.tile as tile
from concourse import bacc
from concourse.bass import ds, ts
from concourse.bass_utils import run_bass_kernel_spmd

B, T, D = 4, 2048, 1024
H, HD = 16, 64
NCORES = 8
HH = H // 2            # heads per core = 8
DH = HH * HD           # head dims per core = 512
QC = 512               # query block (attention outer tile)
NQC = T // QC          # 4 query blocks
KT = 128               # key tile
TB = 512               # token block for projections
F32 = mybir.dt.float32
F32R = mybir.dt.float32r
BF16 = mybir.dt.bfloat16

# static engine cost estimates (ns) used only to pace filler emission
PE_NS = 1e9 / 2.4e9
ACT_NS = 1e9 / 1.2e9
ACT_OH = float(os.environ.get("KERNEL_ACT_OH", "290"))
THRESH = float(os.environ.get("KERNEL_THRESH", "300"))
TXP_INLINE = os.environ.get("KERNEL_TXP_INLINE", "0") == "1"

LAST_RESULTS = None


def _build_nc():
    nc = bacc.Bacc(
        "TRN2",
        target_bir_lowering=False,
        debug=False,
        enable_asserts=False,
        num_devices=NCORES,
    )

    xT = nc.dram_tensor("xT", [D, T], BF16, kind="ExternalInput").ap()
    wq = nc.dram_tensor("wq", [D, DH], BF16, kind="ExternalInput").ap()
    wk = nc.dram_tensor("wk", [D, DH], BF16, kind="ExternalInput").ap()
    wv = nc.dram_tensor("wv", [D, DH], BF16, kind="ExternalInput").ap()
    wo = nc.dram_tensor("wo", [DH, D], BF16, kind="ExternalInput").ap()
    bq_t = nc.dram_tensor("bq_t", [128, DH // 128], F32, kind="ExternalInput").ap()
    bk_t = nc.dram_tensor("bk_t", [128, DH // 128], F32, kind="ExternalInput").ap()
    bo_t = nc.dram_tensor("bo_t", [128, D // 128], F32, kind="ExternalInput").ap()
    tri = nc.dram_tensor("tri", [128, 128], BF16, kind="ExternalInput").ap()
    ident = nc.dram_tensor("ident", [128, 128], BF16, kind="ExternalInput").ap()
    outT = nc.dram_tensor("outT", [D, T], BF16, kind="ExternalOutput").ap()
    dbg = os.environ.get("KERNEL_DEBUG", "0") == "1"
    if dbg:
        qt_d = nc.dram_tensor("qt_d", [128, 4 * T], BF16, kind="ExternalOutput").ap()
        kt_d = nc.dram_tensor("kt_d", [128, 4 * T], BF16, kind="ExternalOutput").ap()
        v_d = nc.dram_tensor("v_d", [128, (T // KT) * HH * (HD + 1)], BF16, kind="ExternalOutput").ap()
        aoT_d = nc.dram_tensor("aoT_d", [128, 4 * T], BF16, kind="ExternalOutput").ap()

    with tile.TileContext(nc) as tc, ExitStack() as ctx:
        res = ctx.enter_context(tc.tile_pool(name="res", bufs=1))
        # resident SBUF tensors; row c*128+p of qt/kt = local dout
        qt_sb = res.tile([128, 4, T], BF16, tag="qt")
        kt_sb = res.tile([128, 4, T], BF16, tag="kt")
        v_sb = res.tile([128, T // KT, HH, HD + 1], BF16, tag="v")
        aoT_sb = res.tile([128, 4, T], BF16, tag="aoT")
        wq_sb = res.tile([128, 8, DH], BF16, tag="wq")
        wk_sb = res.tile([128, 8, DH], BF16, tag="wk")
        wv_sb = res.tile([128, 8, DH], BF16, tag="wv")
        wo_sb = res.tile([128, 4, D], BF16, tag="wo")
        tri_sb = res.tile([128, 128], BF16, tag="tri")
        id_sb = res.tile([128, 128], BF16, tag="id")
        bq_sb = res.tile([128, 4], F32, tag="bq")
        bk_sb = res.tile([128, 4], F32, tag="bk")
        bo_sb = res.tile([128, 8], F32, tag="bo")

        # PSUM: sc2 2 banks x2 + pv 2 banks x1 + mm 1 bank x2 = 8 banks
        sc2p = ctx.enter_context(tc.tile_pool(name="sc2", bufs=2, space="PSUM"))
        pvp = ctx.enter_context(tc.tile_pool(name="pvp", bufs=1, space="PSUM"))
        mm = ctx.enter_context(tc.tile_pool(name="mm", bufs=2, space="PSUM"))
        xpool = ctx.enter_context(tc.tile_pool(name="xp", bufs=2))
        # et is a ring holding every exp tile of the current (qc, hp) block:
        # PV accumulation groups must run back-to-back per PSUM bank, so PV
        # for chunk cc is deferred until its diagonal tile and then reads all
        # earlier exp tiles.  16 live tiles at qc=3 + slack for the next hp.
        etp = ctx.enter_context(tc.tile_pool(name="et", bufs=18))
        pvr = ctx.enter_context(tc.tile_pool(name="pvr", bufs=3))
        rdp = ctx.enter_context(tc.tile_pool(name="rd", bufs=3))
        stp = ctx.enter_context(tc.tile_pool(name="st", bufs=5))
        # ao tiles live only from normalize until the (qc, hp) transpose
        aop = ctx.enter_context(tc.tile_pool(name="aop", bufs=8))

        # initial DMAs: the SP queue needs ~650ns PER dma_start issue (the
        # transfers are only ~364ns), so batch chunks in pairs and issue on
        # two queues in parallel (SP: wq,wk,biases; ACT: x0,wv,tri,id)
        xts = {}
        xTv = xT.rearrange("(c p) t -> p c t", p=128)
        wqv = wq.rearrange("(c p) d -> p c d", p=128)
        wkv = wk.rearrange("(c p) d -> p c d", p=128)
        wvv = wv.rearrange("(c p) d -> p c d", p=128)

        def load_x(tb):
            xt = xpool.tile([128, 8, TB], BF16, tag="xt", name="xt")
            for c4 in range(2):
                nc.sync.dma_start(
                    out=xt[:, 4 * c4 : 4 * c4 + 4, :],
                    in_=xTv[:, 4 * c4 : 4 * c4 + 4, ts(tb, TB)],
                )
            xts[tb] = xt

        xt0 = xpool.tile([128, 8, TB], BF16, tag="xt", name="xt")
        for c2 in range(4):
            sl2 = slice(2 * c2, 2 * c2 + 2)
            nc.sync.dma_start(out=wq_sb[:, sl2, :], in_=wqv[:, sl2, :])
            nc.scalar.dma_start(out=xt0[:, sl2, :], in_=xTv[:, sl2, ts(0, TB)])
        xts[0] = xt0
        for c2 in range(4):
            sl2 = slice(2 * c2, 2 * c2 + 2)
            nc.sync.dma_start(out=wk_sb[:, sl2, :], in_=wkv[:, sl2, :])
            nc.scalar.dma_start(out=wv_sb[:, sl2, :], in_=wvv[:, sl2, :])
        nc.sync.dma_start(out=bq_sb, in_=bq_t)
        nc.sync.dma_start(out=bk_sb, in_=bk_t)
        nc.sync.dma_start(out=bo_sb, in_=bo_t)
        nc.scalar.dma_start(out=tri_sb, in_=tri)
        nc.scalar.dma_start(out=id_sb, in_=ident)
        nc.vector.memset(v_sb[:, :, :, HD : HD + 1], 1.0)
        wo_loaded = [False]

        def load_wo():
            if not wo_loaded[0]:
                wo_loaded[0] = True
                for c in range(4):
                    nc.sync.dma_start(out=wo_sb[:, c, :], in_=wo[ts(c, 128), :])

        # ---- filler machinery: projq units must land before the next query
        # block; lateq units (out-proj, transposes) have no deadline and are
        # saved for the exp-bound late blocks.
        qq = deque()      # Q units: due before the next query block starts
        projq = deque()   # K/V units: due before the next block's diagonal
        lateq = deque()
        debt = [0.0]  # accumulated ACT-over-PE time not yet filled

        stats = collections.Counter()
        where = ["init"]
        due_count = [0]  # kv units at the head of projq due this block

        def fillers(thresh=THRESH):
            while debt[0] > thresh:
                if qq:
                    qq.popleft()()
                    stats[(where[0], "qq")] += 1
                elif projq:
                    projq.popleft()()
                    if due_count[0] > 0:
                        due_count[0] -= 1
                    stats[(where[0], "kv")] += 1
                elif lateq:
                    lateq.popleft()()
                    stats[(where[0], "late")] += 1
                else:
                    stats[(where[0], "DRY")] += 1
                    break

        def qk_unit(tb, dt, wsb, bias, dst):
            def emit():
                ps = mm.tile([128, TB], F32, tag="mm", name="ps")
                for c in range(8):
                    nc.tensor.matmul(
                        ps, wsb[:, c, ts(dt, 128)], xts[tb][:, c, :],
                        start=(c == 0), stop=(c == 7),
                    )
                nc.vector.tensor_scalar_add(
                    dst[:, dt, ts(tb, TB)], ps, bias[:, dt : dt + 1]
                )
                debt[0] -= 8 * TB * PE_NS
            return emit

        def v_unit(tb, tt):
            def emit():
                ps = mm.tile([128, DH], F32, tag="mm", name="ps")
                for c in range(8):
                    nc.tensor.matmul(
                        ps, xts[tb][:, c, ts(tt, 128)], wv_sb[:, c, :],
                        start=(c == 0), stop=(c == 7),
                    )
                nc.vector.tensor_copy(
                    v_sb[:, tb * (TB // 128) + tt, :, 0:HD],
                    ps.rearrange("p (h c) -> p h c", h=HH),
                )
                debt[0] -= 8 * DH * PE_NS
            return emit

        def q_units(tb):
            return [qk_unit(tb, dt, wq_sb, bq_sb, qt_sb) for dt in range(4)]

        def kv_units(tb):
            return [qk_unit(tb, dt, wk_sb, bk_sb, kt_sb) for dt in range(4)] + [
                v_unit(tb, tt) for tt in range(4)
            ]

        def txp_unit(qc, hp, aot):
            def emit():
                tx = mm.tile([128, 4, 128], BF16, tag="mm", name="tx")
                for cc in range(4):
                    nc.tensor.transpose(tx[:, cc, :], aot[:, cc, :], id_sb)
                nc.vector.tensor_copy(
                    aoT_sb[:, hp, ts(qc, QC)], tx.rearrange("p a b -> p (a b)")
                )
                debt[0] -= 4 * 128 * PE_NS
            return emit

        def oproj_unit(qc, dt):
            def emit():
                if qc == 3 and dt % 2 == 1:
                    ps = sc2p.tile([128, 2, QC], F32, tag="sc", name="ps")[:, 0, :]
                else:
                    ps = mm.tile([128, QC], F32, tag="mm", name="ps")
                for hp in range(4):
                    nc.tensor.matmul(ps, wo_sb[:, hp, ts(dt, 128)],
                                     aoT_sb[:, hp, ts(qc, QC)],
                                     start=(hp == 0), stop=(hp == 3))
                st = stp.tile([128, QC], BF16, tag="st", name="st")
                if qc == 3:
                    # tail: the Activation engine is idle once attention is
                    # done; Identity supports a per-partition bias AP
                    nc.scalar.activation(
                        st, ps, mybir.ActivationFunctionType.Identity,
                        bias=bo_sb[:, dt : dt + 1],
                    )
                else:
                    nc.vector.tensor_scalar_add(st, ps, bo_sb[:, dt : dt + 1])
                nc.sync.dma_start(out=outT[ts(dt, 128), ts(qc, QC)], in_=st)
                debt[0] -= 4 * QC * PE_NS
            return emit

        # token block 0 projections, chunk-major across four concurrent
        # PSUM groups (2 in mm + 2 in sc2) so the tensor engine keeps pace
        # with the streaming weight/x DMAs instead of stalling per group
        def proj_tb0():
            xt = xts[0]
            for kind in ("q", "k", "v"):
                pa = mm.tile([128, TB], F32, tag="mm", name="pa")
                pb = mm.tile([128, TB], F32, tag="mm", name="pb")
                sc = sc2p.tile([128, 2, QC], F32, tag="sc", name="sc")
                pss = [pa, pb, sc[:, 0, :], sc[:, 1, :]]
                for c in range(8):
                    for g in range(4):
                        if kind == "q":
                            nc.tensor.matmul(pss[g], wq_sb[:, c, ts(g, 128)],
                                             xt[:, c, :], start=(c == 0), stop=(c == 7))
                        elif kind == "k":
                            nc.tensor.matmul(pss[g], wk_sb[:, c, ts(g, 128)],
                                             xt[:, c, :], start=(c == 0), stop=(c == 7))
                        else:
                            nc.tensor.matmul(pss[g], xt[:, c, ts(g, 128)],
                                             wv_sb[:, c, :], start=(c == 0), stop=(c == 7))
                for g in range(4):
                    if kind == "q":
                        if g % 2 == 0:
                            nc.vector.tensor_scalar_add(qt_sb[:, g, ts(0, TB)],
                                                        pss[g], bq_sb[:, g : g + 1])
                        else:
                            nc.scalar.activation(
                                qt_sb[:, g, ts(0, TB)], pss[g],
                                mybir.ActivationFunctionType.Identity,
                                bias=bq_sb[:, g : g + 1])
                    elif kind == "k":
                        if g % 2 == 0:
                            nc.vector.tensor_scalar_add(kt_sb[:, g, ts(0, TB)],
                                                        pss[g], bk_sb[:, g : g + 1])
                        else:
                            nc.scalar.activation(
                                kt_sb[:, g, ts(0, TB)], pss[g],
                                mybir.ActivationFunctionType.Identity,
                                bias=bk_sb[:, g : g + 1])
                    else:
                        eng = nc.vector if g % 2 == 0 else nc.scalar
                        if g % 2 == 0:
                            nc.vector.tensor_copy(
                                v_sb[:, g, :, 0:HD],
                                pss[g].rearrange("p (h c) -> p h c", h=HH))
                        else:
                            nc.scalar.activation(
                                v_sb[:, g, :, 0:HD],
                                pss[g].rearrange("p (h c) -> p h c", h=HH),
                                mybir.ActivationFunctionType.Copy)

        proj_tb0()
        debt[0] = 0.0

        for qc in range(NQC):
            due_count[0] = len(projq)  # leftovers: due at this diagonal
            if qc + 1 < NQC:
                load_x(qc + 1)
                qq.extend(q_units(qc + 1))
                projq.extend(kv_units(qc + 1))
            nkt = (qc + 1) * (QC // KT)
            for hp in range(4):
                where[0] = f"qc{qc}hp{hp}"
                # chunk stride padded to 128 floats so each 65-float accumulation
                # region stays inside one 2 KiB PSUM bank (head i -> bank i)
                pv = pvp.tile([128, 2, 4, 128], F32, tag="pv", name="pv")
                ets = []
                for kt in range(nkt):
                    j = kt - qc * (QC // KT)  # >= 0 on the diagonal block
                    q0 = j * KT if j > 0 else 0
                    qn = QC - q0
                    if j == 0 and hp == 0 and due_count[0] > 0:
                        # this block's K/V must be emitted before the first
                        # diagonal QK reads them (engines run in order)
                        while due_count[0] > 0:
                            projq.popleft()()
                            due_count[0] -= 1
                            stats[(where[0], "kv-forced")] += 1
                        debt[0] = min(debt[0], 0.0)
                    sc = sc2p.tile([128, 2, QC], F32, tag="sc", name="sc")
                    for i in range(2):
                        po = i * 64
                        nc.tensor.matmul(
                            sc[:, i, q0:],
                            kt_sb[po : po + 64, hp, ts(kt, 128)],
                            qt_sb[po : po + 64, hp, ds(qc * QC + q0, qn)],
                            start=True,
                            stop=True,
                        )
                    et = etp.tile([128, 2, QC], BF16, tag="et", name="et")
                    nc.scalar.activation(
                        et[:, :, q0:],
                        sc[:, :, q0:],
                        mybir.ActivationFunctionType.Exp,
                        scale=0.125,
                    )
                    ets.append(et)
                    pe_tile = 2 * qn * PE_NS
                    if j >= 0:
                        # causal boundary lies inside q-chunk j only; the
                        # mask is all-ones for chunks right of it
                        for i in range(2):
                            nc.vector.tensor_mul(
                                et[:, i, ts(j, KT)], et[:, i, ts(j, KT)], tri_sb
                            )
                        # chunk j's keys are complete: emit its whole PV
                        # accumulation group back-to-back (one per bank)
                        cc = j
                        for i in range(2):
                            for kk in range(kt + 1):
                                nc.tensor.matmul(
                                    pv[:, i, cc, 0 : HD + 1],
                                    ets[kk][:, i, ts(cc, 128)],
                                    v_sb[:, kk, hp * 2 + i, :],
                                    start=(kk == 0),
                                    stop=(kk == kt),
                                )
                        pe_tile += 2 * (kt + 1) * 65 * PE_NS
                    debt[0] += (2 * qn * ACT_NS + ACT_OH) - pe_tile
                    fillers()
                    if qc == 3 and hp >= 2 and kt % (4 - hp) == 0 and not (
                        qq or projq
                    ) and lateq:
                        # the static debt model under-pops here; these
                        # stretches are exp-bound with plenty queued
                        lateq.popleft()()
                        stats[(where[0], "late-forced")] += 1
                # drain pv quickly so the single PSUM slot frees: exact
                # denominators from PSUM, raw bf16 copy, then normalize
                rd = rdp.tile([128, 2, 4], F32, tag="rd", name="rd")
                nc.vector.reciprocal(rd, pv[:, :, :, HD : HD + 1])
                praw = pvr.tile([128, 2, 4, HD], BF16, tag="praw", name="praw")
                nc.vector.tensor_copy(praw, pv[:, :, :, 0:HD])
                aot = aop.tile([128, 4, 128], BF16, tag="ao", name="aot")
                for i in range(2):
                    for cc in range(4):
                        nc.vector.tensor_scalar_mul(
                            aot[:, cc, i * 64 : (i + 1) * 64],
                            praw[:, i, cc, :],
                            rd[:, i, cc : cc + 1],
                        )
                if TXP_INLINE:
                    txp_unit(qc, hp, aot)()
                else:
                    lateq.append(txp_unit(qc, hp, aot))
            # Q of the next block must be in before it starts; K/V only
            # before its diagonal tiles, so they keep filling the next block
            where[0] = f"qc{qc}end"
            while qq:
                qq.popleft()()
                stats[(where[0], "qq-forced")] += 1
            debt[0] = min(debt[0], 0.0)
            load_wo()
            for dt in range(8):
                lateq.append(oproj_unit(qc, dt))
        where[0] = "tail"
        while lateq:
            lateq.popleft()()
            stats[("tail", "late")] += 1
        if os.environ.get("KERNEL_STATS", "0") == "1":
            for k in sorted(stats):
                print(f"  {k}: {stats[k]}")
        if dbg:
            nc.sync.dma_start(out=qt_d, in_=qt_sb.rearrange("p a b -> p (a b)"))
            nc.sync.dma_start(out=kt_d, in_=kt_sb.rearrange("p a b -> p (a b)"))
            nc.sync.dma_start(out=v_d, in_=v_sb.rearrange("p a b c -> p (a b c)"))
            nc.sync.dma_start(out=aoT_d, in_=aoT_sb.rearrange("p a b -> p (a b)"))

    nc.compile()
    return nc


def kernel(x, Wq, bq, Wk, bk, Wv, bv, Wo, bo):
    global LAST_RESULTS
    import ml_dtypes

    x = np.asarray(x, np.float32)
    Wq, bq = np.asarray(Wq, np.float32), np.asarray(bq, np.float32)
    Wk, bk = np.asarray(Wk, np.float32), np.asarray(bk, np.float32)
    Wv, bv = np.asarray(Wv, np.float32), np.asarray(bv, np.float32)
    Wo, bo = np.asarray(Wo, np.float32), np.asarray(bo, np.float32)

    tri = np.triu(np.ones((128, 128), np.float32)).astype(ml_dtypes.bfloat16)
    ident = np.eye(128, dtype=np.float32).astype(ml_dtypes.bfloat16)

    in_maps = []
    for c in range(NCORES):
        b, hh = c // 2, c % 2
        sl = slice(hh * DH, (hh + 1) * DH)
        # attn out includes +bv per head dim (softmax weights sum to 1), so
        # bv contributes bv_slice @ Wo_slice to this core's partial output;
        # bo itself is carried by the hh == 0 core of each pair.
        bo_eff = bv[sl] @ Wo[sl, :] + (bo if hh == 0 else 0.0)
        in_maps.append(
            {
                "xT": np.ascontiguousarray(x[b].T).astype(ml_dtypes.bfloat16),
                "wq": np.ascontiguousarray(Wq[:, sl]).astype(ml_dtypes.bfloat16),
                "wk": np.ascontiguousarray(Wk[:, sl]).astype(ml_dtypes.bfloat16),
                "wv": np.ascontiguousarray(Wv[:, sl]).astype(ml_dtypes.bfloat16),
                "wo": np.ascontiguousarray(Wo[sl, :]).astype(ml_dtypes.bfloat16),
                "bq_t": np.ascontiguousarray(bq[sl].reshape(4, 128).T),
                "bk_t": np.ascontiguousarray(bk[sl].reshape(4, 128).T),
                "bo_t": np.ascontiguousarray(bo_eff.reshape(8, 128).T),
                "tri": tri,
                "ident": ident,
            }
        )

    nc = _build_nc()
    res = run_bass_kernel_spmd(
        nc,
        in_maps,
        core_ids=list(range(NCORES)),
        trace=bool(int(os.environ.get("KERNEL_TRACE", "0"))),
    )
    LAST_RESULTS = res

    out = np.empty((B, T, D), np.float32)
    for b in range(B):
        acc = np.asarray(res.results[2 * b]["outT"], np.float32) + np.asarray(
            res.results[2 * b + 1]["outT"], np.float32
        )
        out[b] = acc.T
    return out
